# revision 4
# baseline (speedup 1.0000x reference)
"""Grouped-query attention, tensor-parallel over heads across 8 TRN2 NeuronCores.

Problem (hardcoded): x[1,1024,4096] @ Wq/Wk/Wv -> RoPE -> causal GQA
(32 q heads, 8 kv groups, head_dim 128) -> out proj Wo -> [1,1024,4096].

Sharding: core r owns q heads 4r..4r+3 and kv group r (Wq/Wk/Wv column
shards, Wo row shard). Each core computes a full [1024,4096] partial of
the output projection; the host sums the 8 partials (the "all-reduce").

Device kernel (per core): the big GEMMs (Q/K/V projections, out-proj)
run in fp8e4 DoubleRow mode, which processes two 128-deep contraction
chunks per instruction at half the cycles/row of bf16.  Precision is
recovered with a 3-term hi/lo split quantization (x_hi@W_hi + x_lo@W_hi
+ x_hi@W_lo), where hi and lo shares one power-of-2 scale so all terms
accumulate in a single PSUM chain; measured end-to-end error matches
bf16.  The attention core (scores, exp, denominators, ctx) stays bf16
with 256-wide s-blocks and causal tile skipping.
"""

import numpy as np
import ml_dtypes

import concourse.bass as bass
import concourse.bacc as bacc
import concourse.mybir as mybir
import concourse.tile as tile
from concourse.bass_utils import run_bass_kernel_spmd

S = 1024          # sequence length
D = 4096          # model dim
H = 32            # query heads (global)
G = 8             # kv groups (global)
HD = 128          # head dim
N_CORES = 8
HPC = H // N_CORES   # 4 query heads per core
QW = HPC * HD        # 512 q-proj cols per core
NDC = D // 128       # 32 contraction chunks
NP = NDC // 2        # 16 DoubleRow chunk pairs
BF = mybir.dt.bfloat16
F8 = mybir.dt.float8e4
F32 = mybir.dt.float32
DR = mybir.MatmulPerfMode.DoubleRow

# quantization scales (powers of 2; hi and lo share the scale so every
# 3-term matmul accumulates in one PSUM chain)
XS = 16.0
WQS = 8192.0        # applied to Wq/sqrt(HD)
WKS = 1024.0
WVS = 1024.0
WOS = 1024.0
CTXS = 16.0
EXP_SHIFT = -6.0    # exp(s - 6): keeps bf16 P comfortably in range

_CACHE = {}


def _build():
    nc = bacc.Bacc("TRN2", target_bir_lowering=False, debug=False,
                   num_devices=N_CORES)

    xh = nc.dram_tensor("xh", [128, NDC, S], F8, kind="ExternalInput")
    xl = nc.dram_tensor("xl", [128, NDC, S], F8, kind="ExternalInput")
    wqh = nc.dram_tensor("wqh", [128, NDC, QW], F8, kind="ExternalInput")
    wql = nc.dram_tensor("wql", [128, NDC, QW], F8, kind="ExternalInput")
    wkh = nc.dram_tensor("wkh", [128, NDC, HD], F8, kind="ExternalInput")
    wkl = nc.dram_tensor("wkl", [128, NDC, HD], F8, kind="ExternalInput")
    wvh = nc.dram_tensor("wvh", [128, NDC, HD], F8, kind="ExternalInput")
    wvl = nc.dram_tensor("wvl", [128, NDC, HD], F8, kind="ExternalInput")
    woh = nc.dram_tensor("woh", [128, HPC, D], F8, kind="ExternalInput")
    wol = nc.dram_tensor("wol", [128, HPC, D], F8, kind="ExternalInput")
    cosT = nc.dram_tensor("cosT", [HD, S], BF, kind="ExternalInput")
    sinT = nc.dram_tensor("sinT", [HD, S], BF, kind="ExternalInput")
    rmat = nc.dram_tensor("rmat", [HD, HD], BF, kind="ExternalInput")
    masks = nc.dram_tensor("masks", [128, 512], BF, kind="ExternalInput")
    out = nc.dram_tensor("out", [S, D], BF, kind="ExternalOutput")

    with tile.TileContext(nc) as tc:
        _emit(tc, nc, xh, xl, wqh, wql, wkh, wkl, wvh, wvl, woh, wol,
              cosT, sinT, rmat, masks, out)
    nc.compile()
    return nc


def _emit(tc, nc, xh, xl, wqh, wql, wkh, wkl, wvh, wvl, woh, wol,
          cosT, sinT, rmat, masks, out):
    import contextlib
    ctx = contextlib.ExitStack()
    with ctx:
        const = ctx.enter_context(tc.tile_pool(name="const", bufs=1))
        work = ctx.enter_context(tc.tile_pool(name="work", bufs=1))
        tmp = ctx.enter_context(tc.tile_pool(name="tmp", bufs=4))
        pt_pool = ctx.enter_context(tc.tile_pool(name="pt", bufs=6))
        outp = ctx.enter_context(tc.tile_pool(name="outp", bufs=4))
        ps = ctx.enter_context(tc.tile_pool(name="ps", bufs=8, space="PSUM"))

        # ---- constants / weights into SBUF ----
        # Leading transfers small so the first k/q chains unblock fast.
        rmat_sb = const.tile([HD, HD], BF, tag="rmat")
        ones_sb = const.tile([128, 1], BF, tag="ones")
        nc.vector.memset(ones_sb[:], 1.0)
        ebias = const.tile([128, 1], F32, tag="ebias")
        nc.vector.memset(ebias[:], EXP_SHIFT)
        wk_sb = const.tile([128, 2, NDC, HD], F8, tag="wk")   # dim1: hi/lo
        nc.sync.dma_start(out=wk_sb[:, 0, 0:4, :], in_=wkh.ap()[:, 0:4, :])
        gxh, gxl = {}, {}
        gqh, gql = {}, {}
        for c in range(0, NDC, 2):
            g = const.tile([128, 2, S], F8, tag=f"xh{c//2}", name=f"xh{c//2}")
            nc.sync.dma_start(out=g[:], in_=xh.ap()[:, c:c + 2, :])
            gxh[c] = g
            if c % 4 == 0:
                g = const.tile([128, 4, QW], F8, tag=f"qh{c//4}", name=f"qh{c//4}")
                nc.sync.dma_start(out=g[:], in_=wqh.ap()[:, c:c + 4, :])
                gqh[c], gqh[c + 2] = g[:, 0:2, :], g[:, 2:4, :]
            if c == 2:
                nc.sync.dma_start(out=wk_sb[:, 0, 4:, :], in_=wkh.ap()[:, 4:, :])
                nc.sync.dma_start(out=wk_sb[:, 1, :, :], in_=wkl.ap())
                nc.sync.dma_start(out=rmat_sb[:], in_=rmat.ap())
        for c in range(0, NDC, 2):
            g = const.tile([128, 2, S], F8, tag=f"xl{c//2}", name=f"xl{c//2}")
            nc.sync.dma_start(out=g[:], in_=xl.ap()[:, c:c + 2, :])
            gxl[c] = g
            if c % 4 == 0:
                g = const.tile([128, 4, QW], F8, tag=f"ql{c//4}", name=f"ql{c//4}")
                nc.sync.dma_start(out=g[:], in_=wql.ap()[:, c:c + 4, :])
                gql[c], gql[c + 2] = g[:, 0:2, :], g[:, 2:4, :]
        cos_sb = const.tile([HD, S], BF, tag="cos")
        nc.sync.dma_start(out=cos_sb[:], in_=cosT.ap())
        sin_sb = const.tile([HD, S], BF, tag="sin")
        nc.sync.dma_start(out=sin_sb[:], in_=sinT.ap())
        wv_sb = const.tile([128, 2, NDC, HD], F8, tag="wv")
        nc.sync.dma_start(out=wv_sb[:, 0, :, :], in_=wvh.ap())
        nc.sync.dma_start(out=wv_sb[:, 1, :, :], in_=wvl.ap())
        mask_sb = const.tile([128, 512], BF, tag="mask")
        nc.sync.dma_start(out=mask_sb[:], in_=masks.ap())
        wo_sb = const.tile([128, 2, HPC, D], F8, tag="wo")    # dim1: hi/lo
        for n in range(2):
            sl = slice(n * 2048, (n + 1) * 2048)
            nc.sync.dma_start(out=wo_sb[:, 0, :, sl], in_=woh.ap()[:, :, sl])
            nc.sync.dma_start(out=wo_sb[:, 1, :, sl], in_=wol.ap()[:, :, sl])

        # persistent activations
        khat = work.tile([HD, S], BF, tag="khat")
        qhat = [work.tile([HD, S], BF, tag=f"qhat{h}", name=f"qhat{h}")
                for h in range(HPC)]
        v_sb = [work.tile([128, HD], BF, tag=f"v{i}", name=f"v{i}")
                for i in range(8)]
        # fp8 hi/lo ctx, head-pairs interleaved on dim1 for DoubleRow
        ctx_hi = [work.tile([128, 2, S], F8, tag=f"cth{u}", name=f"cth{u}")
                  for u in range(2)]
        ctx_lo = [work.tile([128, 2, S], F8, tag=f"ctl{u}", name=f"ctl{u}")
                  for u in range(2)]

        # ---- fp8 DoubleRow 3-term projection chain ----
        def proj_chain(pp, wpair_hi, wpair_lo, ssl, first, last):
            """Accumulate one [128,256] psum over 16 chunk pairs x 3 terms."""
            for p in range(NP):
                c = 2 * p
                nc.tensor.matmul(pp[:], wpair_hi(c), gxh[c][:, :, ssl],
                                 start=(first and p == 0), stop=False,
                                 perf_mode=DR)
            for p in range(NP):
                c = 2 * p
                nc.tensor.matmul(pp[:], wpair_hi(c), gxl[c][:, :, ssl],
                                 start=False, stop=False, perf_mode=DR)
            for p in range(NP):
                c = 2 * p
                nc.tensor.matmul(pp[:], wpair_lo(c), gxh[c][:, :, ssl],
                                 start=False, stop=(last and p == NP - 1),
                                 perf_mode=DR)

        def rope(dst, raws, descale):
            """raws: 4 per-s-chunk psums; evacuate, rotate, combine."""
            raw = tmp.tile([HD, S], BF, tag="rope_raw", name="rope_raw", bufs=2)
            for sc4 in range(4):
                nc.scalar.activation(raw[:, sc4 * 256:(sc4 + 1) * 256],
                                     raws[sc4][:],
                                     mybir.ActivationFunctionType.Copy,
                                     scale=descale)
            t1 = tmp.tile([HD, S], BF, tag="rope_t1", name="rope_t1", bufs=2)
            nc.vector.tensor_mul(t1[:], raw[:], cos_sb[:])
            for hf in range(2):
                sl = slice(hf * 512, (hf + 1) * 512)
                rq = ps.tile([HD, 512], F32, tag="ps", name="rq")
                nc.tensor.matmul(rq[:], rmat_sb[:], raw[:, sl],
                                 start=True, stop=True)
                rqs = tmp.tile([HD, 512], BF, tag="rope_rqs", name="rope_rqs",
                               bufs=2)
                nc.scalar.activation(rqs[:], rq[:],
                                     mybir.ActivationFunctionType.Copy)
                t2 = tmp.tile([HD, 512], BF, tag="rope_t2", name="rope_t2",
                              bufs=2)
                nc.vector.tensor_mul(t2[:], rqs[:], sin_sb[:, sl])
                nc.vector.tensor_add(dst[:, sl], t1[:, sl], t2[:])

        # K projection + rope
        kraws = []
        for sc4 in range(4):
            ssl = slice(sc4 * 256, (sc4 + 1) * 256)
            kp = ps.tile([128, 256], F32, tag="ps", name="kp")
            proj_chain(kp, lambda c: wk_sb[:, 0, c:c + 2, :],
                       lambda c: wk_sb[:, 1, c:c + 2, :], ssl, True, True)
            kraws.append(kp)
        rope(khat, kraws, 1.0 / (XS * WKS))

        # Q projections + rope
        for h in range(HPC):
            hsl = slice(h * HD, (h + 1) * HD)
            qraws = []
            for sc4 in range(4):
                ssl = slice(sc4 * 256, (sc4 + 1) * 256)
                qp = ps.tile([128, 256], F32, tag="ps", name="qp")
                proj_chain(qp, lambda c: gqh[c][:, :, hsl],
                           lambda c: gql[c][:, :, hsl], ssl, True, True)
                qraws.append(qp)
            rope(qhat[h], qraws, 1.0 / (XS * WQS))

        # V projection: out [128 tok-chunk, HD]; x pairs are the stationary
        # side so v lands token-major, ready to be ctx's lhsT.
        for i in range(8):
            tsl = slice(i * 128, (i + 1) * 128)
            vp = ps.tile([128, HD], F32, tag="ps", name="vp")
            for p in range(NP):
                c = 2 * p
                nc.tensor.matmul(vp[:], gxh[c][:, :, tsl], wv_sb[:, 0, c:c + 2, :],
                                 start=(p == 0), stop=False, perf_mode=DR)
            for p in range(NP):
                c = 2 * p
                nc.tensor.matmul(vp[:], gxl[c][:, :, tsl], wv_sb[:, 0, c:c + 2, :],
                                 start=False, stop=False, perf_mode=DR)
            for p in range(NP):
                c = 2 * p
                nc.tensor.matmul(vp[:], gxh[c][:, :, tsl], wv_sb[:, 1, c:c + 2, :],
                                 start=False, stop=(p == NP - 1), perf_mode=DR)
            nc.scalar.activation(v_sb[i][:], vp[:],
                                 mybir.ActivationFunctionType.Copy,
                                 scale=1.0 / (XS * WVS))

        # ---- attention: bf16, [t,s] layout, 256-wide s-blocks ----
        for h in range(HPC):
            for b in range(4):
                ssl = slice(b * 256, (b + 1) * 256)
                npair = b + 1
                pts = []
                for tp in range(npair):
                    st = ps.tile([128, 512], F32, tag="ps", name="st")
                    for i in range(2):
                        t0 = (2 * tp + i) * 128
                        nc.tensor.matmul(st[:, i * 256:(i + 1) * 256],
                                         khat[:, t0:t0 + 128], qhat[h][:, ssl],
                                         start=True, stop=True)
                    pt = pt_pool.tile([128, 512], BF, tag="pt", name="pt")
                    nc.scalar.activation(pt[:], st[:],
                                         mybir.ActivationFunctionType.Exp,
                                         bias=ebias[:])
                    if tp == npair - 1:
                        nc.vector.tensor_mul(pt[:], pt[:], mask_sb[:])
                    pts.append(pt)
                den = ps.tile([1, 256], F32, tag="ps", name="den")
                n_mm = 2 * npair
                k = 0
                for pt in pts:
                    for i in range(2):
                        nc.tensor.matmul(den[:], ones_sb[:],
                                         pt[:, i * 256:(i + 1) * 256],
                                         start=(k == 0), stop=(k == n_mm - 1))
                        k += 1
                cx = ps.tile([HD, 256], F32, tag="ps", name="cx")
                k = 0
                for tp, pt in enumerate(pts):
                    for i in range(2):
                        nc.tensor.matmul(cx[:], v_sb[2 * tp + i][:],
                                         pt[:, i * 256:(i + 1) * 256],
                                         start=(k == 0), stop=(k == n_mm - 1))
                        k += 1
                rec = tmp.tile([1, 256], F32, tag="rec", name="rec", bufs=2)
                nc.vector.reciprocal(rec[:], den[:])
                bc = tmp.tile([128, 256], F32, tag="bc", name="bc", bufs=2)
                nc.gpsimd.partition_broadcast(bc[:], rec[:])
                ctxn = tmp.tile([HD, 256], F32, tag="ctxn", name="ctxn", bufs=2)
                nc.vector.scalar_tensor_tensor(
                    ctxn[:], cx[:], CTXS, bc[:],
                    op0=mybir.AluOpType.mult, op1=mybir.AluOpType.mult)
                u, par = divmod(h, 2)
                nc.scalar.activation(ctx_hi[u][:, par, ssl], ctxn[:],
                                     mybir.ActivationFunctionType.Copy)
                nc.vector.tensor_sub(ctx_lo[u][:, par, ssl], ctxn[:],
                                     ctx_hi[u][:, par, ssl])

        # ---- out-proj: fp8 DoubleRow 3-term over head pairs ----
        descale = 1.0 / (CTXS * WOS)
        for t8 in range(8):
            tsl = slice(t8 * 128, (t8 + 1) * 128)
            for n4 in range(4):
                ot = outp.tile([128, 1024], BF, tag="ot", name="ot")
                for sub in range(4):
                    n = 4 * n4 + sub
                    nsl = slice(n * 256, (n + 1) * 256)
                    op = ps.tile([128, 256], F32, tag="ps", name="op")
                    k = 0
                    for u in range(2):
                        for chi, whi in ((ctx_hi, 0), (ctx_lo, 0), (ctx_hi, 1)):
                            nc.tensor.matmul(
                                op[:], chi[u][:, :, tsl],
                                wo_sb[:, whi, 2 * u:2 * u + 2, nsl],
                                start=(k == 0), stop=(k == 5), perf_mode=DR)
                            k += 1
                    eng = nc.vector if (sub % 2 == 0) else nc.gpsimd
                    eng.tensor_scalar_mul(ot[:, sub * 256:(sub + 1) * 256],
                                          op[:], descale)
                nc.sync.dma_start(
                    out=out.ap()[tsl, n4 * 1024:(n4 + 1) * 1024], in_=ot[:])


def _prep_inputs(x, cos, sin, Wq, Wk, Wv, Wo):
    """Host-side shard + hi/lo fp8 quantization. Returns per-core inputs."""
    bf = ml_dtypes.bfloat16
    f8 = ml_dtypes.float8_e4m3

    def hilo(a, s):
        hi = np.asarray(a * s, np.float32).astype(f8)
        lo = (np.asarray(a * s, np.float32) - hi.astype(np.float32)).astype(f8)
        return hi, lo

    x2 = np.asarray(x, np.float32).reshape(S, D)
    xTh = np.ascontiguousarray(x2.T).reshape(NDC, 128, S).transpose(1, 0, 2)
    xh_, xl_ = hilo(np.ascontiguousarray(xTh), XS)

    cosT = np.ascontiguousarray(np.asarray(cos, np.float32).T).astype(bf)
    sinT = np.ascontiguousarray(np.asarray(sin, np.float32).T).astype(bf)

    rmat = np.zeros((HD, HD), np.float32)
    half = HD // 2
    rmat[np.arange(half), np.arange(half) + half] = 1.0
    rmat[np.arange(half) + half, np.arange(half)] = -1.0
    rmat = rmat.astype(bf)

    # diagonal pair mask: keep when t_local (= i*128 + p) <= s_local
    lt = np.arange(128)[:, None]
    ls = np.arange(256)[None, :]
    masks = np.concatenate([(lt + 128 * i <= ls) for i in range(2)], axis=1)
    masks = np.ascontiguousarray(masks).astype(bf)     # [128, 512]

    scale = 1.0 / np.sqrt(np.float32(HD))
    Wq_ = np.asarray(Wq, np.float32) * scale
    Wk_ = np.asarray(Wk, np.float32)
    Wv_ = np.asarray(Wv, np.float32)
    Wo_ = np.asarray(Wo, np.float32)

    def chunked(w):  # [D, m] -> [128, NDC, m]
        m = w.shape[1]
        return np.ascontiguousarray(
            w.reshape(NDC, 128, m).transpose(1, 0, 2))

    in_maps = []
    for r in range(N_CORES):
        wqh_, wql_ = hilo(chunked(Wq_[:, r * QW:(r + 1) * QW]), WQS)
        wkh_, wkl_ = hilo(chunked(Wk_[:, r * HD:(r + 1) * HD]), WKS)
        wvh_, wvl_ = hilo(chunked(Wv_[:, r * HD:(r + 1) * HD]), WVS)
        wo_r = np.ascontiguousarray(
            Wo_[r * QW:(r + 1) * QW, :].reshape(HPC, 128, D)
            .transpose(1, 0, 2))
        woh_, wol_ = hilo(wo_r, WOS)
        in_maps.append({
            "xh": xh_, "xl": xl_, "wqh": wqh_, "wql": wql_,
            "wkh": wkh_, "wkl": wkl_, "wvh": wvh_, "wvl": wvl_,
            "woh": woh_, "wol": wol_,
            "cosT": cosT, "sinT": sinT, "rmat": rmat, "masks": masks,
        })
    return in_maps


def get_nc():
    if "nc" not in _CACHE:
        _CACHE["nc"] = _build()
    return _CACHE["nc"]


def kernel(x, mask, cos, sin, Wq, Wk, Wv, Wo):
    nc = get_nc()
    in_maps = _prep_inputs(x, cos, sin, Wq, Wk, Wv, Wo)
    res = run_bass_kernel_spmd(nc, in_maps, core_ids=list(range(N_CORES)))
    acc = np.zeros((S, D), np.float32)
    for r in range(N_CORES):
        acc += res.results[r]["out"].astype(np.float32)
    return acc[None]


if __name__ == "__main__":
    print("built:", get_nc() is not None)


# revision 6
# speedup vs baseline: 1.0664x; 1.0664x over previous
"""Grouped-query attention, tensor-parallel over heads across 8 TRN2 NeuronCores.

Problem (hardcoded): x[1,1024,4096] @ Wq/Wk/Wv -> RoPE -> causal GQA
(32 q heads, 8 kv groups, head_dim 128) -> out proj Wo -> [1,1024,4096].

Sharding: core r owns q heads 4r..4r+3 and kv group r (Wq/Wk/Wv column
shards, Wo row shard). Each core computes a full [1024,4096] partial of
the output projection; the host sums the 8 partials (the "all-reduce").

Device kernel (per core): the big GEMMs (Q/K/V projections, out-proj)
run in fp8e4 DoubleRow mode, which processes two 128-deep contraction
chunks per instruction at half the cycles/row of bf16.  Precision is
recovered with a 3-term hi/lo split quantization (x_hi@W_hi + x_lo@W_hi
+ x_hi@W_lo), where hi and lo shares one power-of-2 scale so all terms
accumulate in a single PSUM chain; measured end-to-end error matches
bf16.  The attention core (scores, exp, denominators, ctx) stays bf16
with 256-wide s-blocks and causal tile skipping.
"""

import numpy as np
import ml_dtypes

import concourse.bass as bass
import concourse.bacc as bacc
import concourse.mybir as mybir
import concourse.tile as tile
from concourse.bass_utils import run_bass_kernel_spmd

S = 1024          # sequence length
D = 4096          # model dim
H = 32            # query heads (global)
G = 8             # kv groups (global)
HD = 128          # head dim
N_CORES = 8
HPC = H // N_CORES   # 4 query heads per core
QW = HPC * HD        # 512 q-proj cols per core
NDC = D // 128       # 32 contraction chunks
NP = NDC // 2        # 16 DoubleRow chunk pairs
BF = mybir.dt.bfloat16
F8 = mybir.dt.float8e4
F32 = mybir.dt.float32
DR = mybir.MatmulPerfMode.DoubleRow

# quantization scales (powers of 2; hi and lo share the scale so every
# 3-term matmul accumulates in one PSUM chain)
XS = 16.0
WQS = 8192.0        # applied to Wq/sqrt(HD)
WKS = 1024.0
WVS = 1024.0
WOS = 1024.0
CTXS = 16.0
EXP_SHIFT = -6.0    # exp(s - 6): keeps bf16 P comfortably in range

_CACHE = {}


def _build():
    nc = bacc.Bacc("TRN2", target_bir_lowering=False, debug=False,
                   num_devices=N_CORES)

    xh = nc.dram_tensor("xh", [128, NDC, S], F8, kind="ExternalInput")
    xl = nc.dram_tensor("xl", [128, NDC, S], F8, kind="ExternalInput")
    wqh = nc.dram_tensor("wqh", [128, NDC, QW], F8, kind="ExternalInput")
    wql = nc.dram_tensor("wql", [128, NDC, QW], F8, kind="ExternalInput")
    wkh = nc.dram_tensor("wkh", [128, NDC, HD], F8, kind="ExternalInput")
    wkl = nc.dram_tensor("wkl", [128, NDC, HD], F8, kind="ExternalInput")
    wvh = nc.dram_tensor("wvh", [128, NDC, HD], F8, kind="ExternalInput")
    wvl = nc.dram_tensor("wvl", [128, NDC, HD], F8, kind="ExternalInput")
    woh = nc.dram_tensor("woh", [128, HPC, D], F8, kind="ExternalInput")
    wol = nc.dram_tensor("wol", [128, HPC, D], F8, kind="ExternalInput")
    cosT = nc.dram_tensor("cosT", [HD, S], BF, kind="ExternalInput")
    sinT = nc.dram_tensor("sinT", [HD, S], BF, kind="ExternalInput")
    rmat = nc.dram_tensor("rmat", [HD, HD], BF, kind="ExternalInput")
    masks = nc.dram_tensor("masks", [128, 512], BF, kind="ExternalInput")
    out = nc.dram_tensor("out", [S, D], BF, kind="ExternalOutput")

    with tile.TileContext(nc) as tc:
        _emit(tc, nc, xh, xl, wqh, wql, wkh, wkl, wvh, wvl, woh, wol,
              cosT, sinT, rmat, masks, out)
    nc.compile()
    return nc


def _emit(tc, nc, xh, xl, wqh, wql, wkh, wkl, wvh, wvl, woh, wol,
          cosT, sinT, rmat, masks, out):
    import contextlib
    ctx = contextlib.ExitStack()
    with ctx:
        const = ctx.enter_context(tc.tile_pool(name="const", bufs=1))
        work = ctx.enter_context(tc.tile_pool(name="work", bufs=1))
        tmp = ctx.enter_context(tc.tile_pool(name="tmp", bufs=4))
        pt_pool = ctx.enter_context(tc.tile_pool(name="pt", bufs=6))
        outp = ctx.enter_context(tc.tile_pool(name="outp", bufs=4))
        ps = ctx.enter_context(tc.tile_pool(name="ps", bufs=8, space="PSUM"))

        # ---- constants / weights into SBUF ----
        # Leading transfers small so the first k/q chains unblock fast.
        rmat_sb = const.tile([HD, HD], BF, tag="rmat")
        ones_sb = const.tile([128, 1], BF, tag="ones")
        nc.vector.memset(ones_sb[:], 1.0)
        ebias = const.tile([128, 1], F32, tag="ebias")
        nc.vector.memset(ebias[:], EXP_SHIFT)
        wk_sb = const.tile([128, 2, NDC, HD], F8, tag="wk")   # dim1: hi/lo
        nc.sync.dma_start(out=wk_sb[:, 0, 0:4, :], in_=wkh.ap()[:, 0:4, :])
        gxh, gxl = {}, {}
        gqh, gql = {}, {}
        for c in range(0, NDC, 2):
            g = const.tile([128, 2, S], F8, tag=f"xh{c//2}", name=f"xh{c//2}")
            nc.sync.dma_start(out=g[:], in_=xh.ap()[:, c:c + 2, :])
            gxh[c] = g
            if c % 4 == 0:
                g = const.tile([128, 4, QW], F8, tag=f"qh{c//4}", name=f"qh{c//4}")
                nc.sync.dma_start(out=g[:], in_=wqh.ap()[:, c:c + 4, :])
                gqh[c], gqh[c + 2] = g[:, 0:2, :], g[:, 2:4, :]
            if c == 2:
                nc.sync.dma_start(out=wk_sb[:, 0, 4:, :], in_=wkh.ap()[:, 4:, :])
                nc.sync.dma_start(out=wk_sb[:, 1, :, :], in_=wkl.ap())
                nc.sync.dma_start(out=rmat_sb[:], in_=rmat.ap())
        for c in range(0, NDC, 2):
            g = const.tile([128, 2, S], F8, tag=f"xl{c//2}", name=f"xl{c//2}")
            nc.sync.dma_start(out=g[:], in_=xl.ap()[:, c:c + 2, :])
            gxl[c] = g
            if c % 4 == 0:
                g = const.tile([128, 4, QW], F8, tag=f"ql{c//4}", name=f"ql{c//4}")
                nc.sync.dma_start(out=g[:], in_=wql.ap()[:, c:c + 4, :])
                gql[c], gql[c + 2] = g[:, 0:2, :], g[:, 2:4, :]
        cos_sb = const.tile([HD, S], BF, tag="cos")
        nc.sync.dma_start(out=cos_sb[:], in_=cosT.ap())
        sin_sb = const.tile([HD, S], BF, tag="sin")
        nc.sync.dma_start(out=sin_sb[:], in_=sinT.ap())
        wv_sb = const.tile([128, 2, NDC, HD], F8, tag="wv")
        nc.sync.dma_start(out=wv_sb[:, 0, :, :], in_=wvh.ap())
        nc.sync.dma_start(out=wv_sb[:, 1, :, :], in_=wvl.ap())
        mask_sb = const.tile([128, 512], BF, tag="mask")
        nc.sync.dma_start(out=mask_sb[:], in_=masks.ap())
        wo_sb = const.tile([128, 2, HPC, D], F8, tag="wo")    # dim1: hi/lo
        for n in range(2):
            sl = slice(n * 2048, (n + 1) * 2048)
            nc.sync.dma_start(out=wo_sb[:, 0, :, sl], in_=woh.ap()[:, :, sl])
            nc.sync.dma_start(out=wo_sb[:, 1, :, sl], in_=wol.ap()[:, :, sl])

        # persistent activations
        khat = work.tile([HD, S], BF, tag="khat")
        qhat = [work.tile([HD, S], BF, tag=f"qhat{h}", name=f"qhat{h}")
                for h in range(HPC)]
        v_sb = [work.tile([128, HD], BF, tag=f"v{i}", name=f"v{i}")
                for i in range(8)]
        # fp8 hi/lo ctx, head-pairs interleaved on dim1 for DoubleRow
        ctx_hi = [work.tile([128, 2, S], F8, tag=f"cth{u}", name=f"cth{u}")
                  for u in range(2)]
        ctx_lo = [work.tile([128, 2, S], F8, tag=f"ctl{u}", name=f"ctl{u}")
                  for u in range(2)]

        # ---- fp8 DoubleRow 3-term projection chain ----
        def proj_chain(pp, wpair_hi, wpair_lo, ssl, first, last):
            """Accumulate one [128,256] psum over 16 chunk pairs x 3 terms."""
            for p in range(NP):
                c = 2 * p
                nc.tensor.matmul(pp[:], wpair_hi(c), gxh[c][:, :, ssl],
                                 start=(first and p == 0), stop=False,
                                 perf_mode=DR)
            for p in range(NP):
                c = 2 * p
                nc.tensor.matmul(pp[:], wpair_hi(c), gxl[c][:, :, ssl],
                                 start=False, stop=False, perf_mode=DR)
            for p in range(NP):
                c = 2 * p
                nc.tensor.matmul(pp[:], wpair_lo(c), gxh[c][:, :, ssl],
                                 start=False, stop=(last and p == NP - 1),
                                 perf_mode=DR)

        def rope_pre(raws, descale):
            """Evacuate psums + cos-mul (Act/DVE work, no PE)."""
            raw = tmp.tile([HD, S], BF, tag="rope_raw", name="rope_raw", bufs=3)
            for sc4 in range(4):
                nc.scalar.activation(raw[:, sc4 * 256:(sc4 + 1) * 256],
                                     raws[sc4][:],
                                     mybir.ActivationFunctionType.Copy,
                                     scale=descale)
            t1 = tmp.tile([HD, S], BF, tag="rope_t1", name="rope_t1", bufs=3)
            nc.vector.tensor_mul(t1[:], raw[:], cos_sb[:])
            return raw, t1

        def rope_mm(raw, hf):
            """One rmat matmul (PE) - emit where PE has slack."""
            sl = slice(hf * 512, (hf + 1) * 512)
            rq = ps.tile([HD, 512], F32, tag="ps", name="rq")
            nc.tensor.matmul(rq[:], rmat_sb[:], raw[:, sl],
                             start=True, stop=True)
            return rq

        def rope_post(dst, t1, rq, hf):
            sl = slice(hf * 512, (hf + 1) * 512)
            rqs = tmp.tile([HD, 512], BF, tag="rope_rqs", name="rope_rqs",
                           bufs=3)
            nc.scalar.activation(rqs[:], rq[:],
                                 mybir.ActivationFunctionType.Copy)
            t2 = tmp.tile([HD, 512], BF, tag="rope_t2", name="rope_t2",
                          bufs=3)
            nc.vector.tensor_mul(t2[:], rqs[:], sin_sb[:, sl])
            nc.vector.tensor_add(dst[:, sl], t1[:, sl], t2[:])

        # K + Q projections with rope pipelined one tensor behind:
        # the rmat matmuls and Act/DVE rope work of tensor t ride inside
        # tensor t+1's projection chains so PE never waits on Act.
        pend = None   # (dst, raw, t1)

        def drain_rope(third):
            nonlocal pend
            if pend is None:
                return
            dst, raw, t1 = pend
            rq0 = rope_mm(raw, 0)
            rq1 = rope_mm(raw, 1)
            rope_post(dst, t1, rq0, 0)
            rope_post(dst, t1, rq1, 1)
            pend = None

        def proj_tensor(dst, whi, wlo, descale):
            nonlocal pend
            raws = []
            for sc4 in range(4):
                ssl = slice(sc4 * 256, (sc4 + 1) * 256)
                pp = ps.tile([128, 256], F32, tag="ps", name="pp")
                proj_chain(pp, whi, wlo, ssl, True, True)
                raws.append(pp)
                if sc4 == 1:
                    drain_rope(False)
            raw, t1 = rope_pre(raws, descale)
            pend = (dst, raw, t1)

        proj_tensor(khat, lambda c: wk_sb[:, 0, c:c + 2, :],
                    lambda c: wk_sb[:, 1, c:c + 2, :], 1.0 / (XS * WKS))
        for h in range(HPC):
            hsl = slice(h * HD, (h + 1) * HD)
            proj_tensor(qhat[h], lambda c, s=hsl: gqh[c][:, :, s],
                        lambda c, s=hsl: gql[c][:, :, s], 1.0 / (XS * WQS))

        # V projection: out [128 tok-chunk, HD]; x pairs are the stationary
        # side so v lands token-major, ready to be ctx's lhsT.
        for i in range(8):
            tsl = slice(i * 128, (i + 1) * 128)
            if i == 1:
                drain_rope(False)
            vp = ps.tile([128, HD], F32, tag="ps", name="vp")
            for p in range(NP):
                c = 2 * p
                nc.tensor.matmul(vp[:], gxh[c][:, :, tsl], wv_sb[:, 0, c:c + 2, :],
                                 start=(p == 0), stop=False, perf_mode=DR)
            for p in range(NP):
                c = 2 * p
                nc.tensor.matmul(vp[:], gxl[c][:, :, tsl], wv_sb[:, 0, c:c + 2, :],
                                 start=False, stop=False, perf_mode=DR)
            for p in range(NP):
                c = 2 * p
                nc.tensor.matmul(vp[:], gxh[c][:, :, tsl], wv_sb[:, 1, c:c + 2, :],
                                 start=False, stop=(p == NP - 1), perf_mode=DR)
            nc.scalar.activation(v_sb[i][:], vp[:],
                                 mybir.ActivationFunctionType.Copy,
                                 scale=1.0 / (XS * WVS))

        # ---- attention + out-proj, software-pipelined ----
        # Stage i+1's score matmuls are emitted before stage i's den/ctx
        # chains so PE has independent work while Act runs stage i's exp.
        # Out-proj token chunks interleave as soon as the s-range of all 4
        # heads' ctx is final (b-major stage order).
        def emit_scores(h, b):
            ssl = slice(b * 256, (b + 1) * 256)
            pts = []
            for tp in range(b + 1):
                st = ps.tile([128, 512], F32, tag="ps", name="st")
                for i in range(2):
                    t0 = (2 * tp + i) * 128
                    nc.tensor.matmul(st[:, i * 256:(i + 1) * 256],
                                     khat[:, t0:t0 + 128], qhat[h][:, ssl],
                                     start=True, stop=True)
                pt = pt_pool.tile([128, 512], BF, tag="pt", name="pt")
                nc.scalar.activation(pt[:], st[:],
                                     mybir.ActivationFunctionType.Exp,
                                     bias=ebias[:])
                if tp == b:
                    nc.vector.tensor_mul(pt[:], pt[:], mask_sb[:])
                pts.append(pt)
            return pts

        def emit_denctx(h, b, pts):
            ssl = slice(b * 256, (b + 1) * 256)
            den = ps.tile([1, 256], F32, tag="ps", name="den")
            n_mm = 2 * (b + 1)
            k = 0
            for pt in pts:
                for i in range(2):
                    nc.tensor.matmul(den[:], ones_sb[:],
                                     pt[:, i * 256:(i + 1) * 256],
                                     start=(k == 0), stop=(k == n_mm - 1))
                    k += 1
            cx = ps.tile([HD, 256], F32, tag="ps", name="cx")
            k = 0
            for tp, pt in enumerate(pts):
                for i in range(2):
                    nc.tensor.matmul(cx[:], v_sb[2 * tp + i][:],
                                     pt[:, i * 256:(i + 1) * 256],
                                     start=(k == 0), stop=(k == n_mm - 1))
                    k += 1
            rec = tmp.tile([1, 256], F32, tag="rec", name="rec", bufs=2)
            nc.vector.reciprocal(rec[:], den[:])
            bc = tmp.tile([128, 256], F32, tag="bc", name="bc", bufs=2)
            nc.gpsimd.partition_broadcast(bc[:], rec[:])
            ctxn = tmp.tile([HD, 256], F32, tag="ctxn", name="ctxn", bufs=2)
            nc.vector.scalar_tensor_tensor(
                ctxn[:], cx[:], CTXS, bc[:],
                op0=mybir.AluOpType.mult, op1=mybir.AluOpType.mult)
            u, par = divmod(h, 2)
            nc.scalar.activation(ctx_hi[u][:, par, ssl], ctxn[:],
                                 mybir.ActivationFunctionType.Copy)
            nc.vector.tensor_sub(ctx_lo[u][:, par, ssl], ctxn[:],
                                 ctx_hi[u][:, par, ssl])

        descale = 1.0 / (CTXS * WOS)

        def emit_outproj(t8):
            tsl = slice(t8 * 128, (t8 + 1) * 128)
            for n4 in range(4):
                ot = outp.tile([128, 1024], BF, tag="ot", name="ot")
                for sub in range(4):
                    n = 4 * n4 + sub
                    nsl = slice(n * 256, (n + 1) * 256)
                    op = ps.tile([128, 256], F32, tag="ps", name="op")
                    k = 0
                    for u in range(2):
                        for chi, whi in ((ctx_hi, 0), (ctx_lo, 0), (ctx_hi, 1)):
                            nc.tensor.matmul(
                                op[:], chi[u][:, :, tsl],
                                wo_sb[:, whi, 2 * u:2 * u + 2, nsl],
                                start=(k == 0), stop=(k == 5), perf_mode=DR)
                            k += 1
                    eng = nc.vector if (sub % 2 == 0) else nc.gpsimd
                    eng.tensor_scalar_mul(ot[:, sub * 256:(sub + 1) * 256],
                                          op[:], descale)
                nc.sync.dma_start(
                    out=out.ap()[tsl, n4 * 1024:(n4 + 1) * 1024], in_=ot[:])

        stages = [(h, b) for b in range(4) for h in range(HPC)]
        prev = None
        outq = []
        for hb in stages:
            pts = emit_scores(*hb)
            if prev is not None:
                (ph, pb), ppts = prev
                emit_denctx(ph, pb, ppts)
                if ph == HPC - 1:
                    outq.extend([2 * pb, 2 * pb + 1])
            if outq:
                emit_outproj(outq.pop(0))
            prev = (hb, pts)
        (ph, pb), ppts = prev
        emit_denctx(ph, pb, ppts)
        outq.extend([2 * pb, 2 * pb + 1])
        for t8 in outq:
            emit_outproj(t8)


def _prep_inputs(x, cos, sin, Wq, Wk, Wv, Wo):
    """Host-side shard + hi/lo fp8 quantization. Returns per-core inputs."""
    bf = ml_dtypes.bfloat16
    f8 = ml_dtypes.float8_e4m3

    def hilo(a, s):
        hi = np.asarray(a * s, np.float32).astype(f8)
        lo = (np.asarray(a * s, np.float32) - hi.astype(np.float32)).astype(f8)
        return hi, lo

    x2 = np.asarray(x, np.float32).reshape(S, D)
    xTh = np.ascontiguousarray(x2.T).reshape(NDC, 128, S).transpose(1, 0, 2)
    xh_, xl_ = hilo(np.ascontiguousarray(xTh), XS)

    cosT = np.ascontiguousarray(np.asarray(cos, np.float32).T).astype(bf)
    sinT = np.ascontiguousarray(np.asarray(sin, np.float32).T).astype(bf)

    rmat = np.zeros((HD, HD), np.float32)
    half = HD // 2
    rmat[np.arange(half), np.arange(half) + half] = 1.0
    rmat[np.arange(half) + half, np.arange(half)] = -1.0
    rmat = rmat.astype(bf)

    # diagonal pair mask: keep when t_local (= i*128 + p) <= s_local
    lt = np.arange(128)[:, None]
    ls = np.arange(256)[None, :]
    masks = np.concatenate([(lt + 128 * i <= ls) for i in range(2)], axis=1)
    masks = np.ascontiguousarray(masks).astype(bf)     # [128, 512]

    scale = 1.0 / np.sqrt(np.float32(HD))
    Wq_ = np.asarray(Wq, np.float32) * scale
    Wk_ = np.asarray(Wk, np.float32)
    Wv_ = np.asarray(Wv, np.float32)
    Wo_ = np.asarray(Wo, np.float32)

    def chunked(w):  # [D, m] -> [128, NDC, m]
        m = w.shape[1]
        return np.ascontiguousarray(
            w.reshape(NDC, 128, m).transpose(1, 0, 2))

    in_maps = []
    for r in range(N_CORES):
        wqh_, wql_ = hilo(chunked(Wq_[:, r * QW:(r + 1) * QW]), WQS)
        wkh_, wkl_ = hilo(chunked(Wk_[:, r * HD:(r + 1) * HD]), WKS)
        wvh_, wvl_ = hilo(chunked(Wv_[:, r * HD:(r + 1) * HD]), WVS)
        wo_r = np.ascontiguousarray(
            Wo_[r * QW:(r + 1) * QW, :].reshape(HPC, 128, D)
            .transpose(1, 0, 2))
        woh_, wol_ = hilo(wo_r, WOS)
        in_maps.append({
            "xh": xh_, "xl": xl_, "wqh": wqh_, "wql": wql_,
            "wkh": wkh_, "wkl": wkl_, "wvh": wvh_, "wvl": wvl_,
            "woh": woh_, "wol": wol_,
            "cosT": cosT, "sinT": sinT, "rmat": rmat, "masks": masks,
        })
    return in_maps


def get_nc():
    if "nc" not in _CACHE:
        _CACHE["nc"] = _build()
    return _CACHE["nc"]


def kernel(x, mask, cos, sin, Wq, Wk, Wv, Wo):
    nc = get_nc()
    in_maps = _prep_inputs(x, cos, sin, Wq, Wk, Wv, Wo)
    res = run_bass_kernel_spmd(nc, in_maps, core_ids=list(range(N_CORES)))
    acc = np.zeros((S, D), np.float32)
    for r in range(N_CORES):
        acc += res.results[r]["out"].astype(np.float32)
    return acc[None]


if __name__ == "__main__":
    print("built:", get_nc() is not None)


# revision 8
# speedup vs baseline: 1.1281x; 1.0578x over previous
"""Grouped-query attention, tensor-parallel over heads across 8 TRN2 NeuronCores.

Problem (hardcoded): x[1,1024,4096] @ Wq/Wk/Wv -> RoPE -> causal GQA
(32 q heads, 8 kv groups, head_dim 128) -> out proj Wo -> [1,1024,4096].

Sharding: core r owns q heads 4r..4r+3 and kv group r (Wq/Wk/Wv column
shards, Wo row shard). Each core computes a full [1024,4096] partial of
the output projection; the host sums the 8 partials (the "all-reduce").

Device kernel (per core): the big GEMMs (Q/K/V projections, out-proj)
run in fp8e4 DoubleRow mode, which processes two 128-deep contraction
chunks per instruction at half the cycles/row of bf16.  Precision is
recovered with a 3-term hi/lo split quantization (x_hi@W_hi + x_lo@W_hi
+ x_hi@W_lo), where hi and lo shares one power-of-2 scale so all terms
accumulate in a single PSUM chain; measured end-to-end error matches
bf16.  The attention core (scores, exp, denominators, ctx) stays bf16
with 256-wide s-blocks and causal tile skipping.
"""

import numpy as np
import ml_dtypes

import concourse.bass as bass
import concourse.bacc as bacc
import concourse.mybir as mybir
import concourse.tile as tile
from concourse.bass_utils import run_bass_kernel_spmd

S = 1024          # sequence length
D = 4096          # model dim
H = 32            # query heads (global)
G = 8             # kv groups (global)
HD = 128          # head dim
N_CORES = 8
HPC = H // N_CORES   # 4 query heads per core
QW = HPC * HD        # 512 q-proj cols per core
NDC = D // 128       # 32 contraction chunks
NP = NDC // 2        # 16 DoubleRow chunk pairs
BF = mybir.dt.bfloat16
F8 = mybir.dt.float8e4
F32 = mybir.dt.float32
DR = mybir.MatmulPerfMode.DoubleRow

# quantization scales (powers of 2; hi and lo share the scale so every
# 3-term matmul accumulates in one PSUM chain)
XS = 16.0
WQS = 8192.0        # applied to Wq/sqrt(HD)
WKS = 1024.0
WVS = 1024.0
WOS = 1024.0
CTXS = 16.0
EXP_SHIFT = -6.0    # exp(s - 6): keeps bf16 P comfortably in range

_CACHE = {}


def _build():
    nc = bacc.Bacc("TRN2", target_bir_lowering=False, debug=False,
                   num_devices=N_CORES)

    xh = nc.dram_tensor("xh", [128, NDC, S], F8, kind="ExternalInput")
    xl = nc.dram_tensor("xl", [128, NDC, S], F8, kind="ExternalInput")
    wqh = nc.dram_tensor("wqh", [128, NDC, QW], F8, kind="ExternalInput")
    wql = nc.dram_tensor("wql", [128, NDC, QW], F8, kind="ExternalInput")
    wkh = nc.dram_tensor("wkh", [128, NDC, HD], F8, kind="ExternalInput")
    wkl = nc.dram_tensor("wkl", [128, NDC, HD], F8, kind="ExternalInput")
    wvh = nc.dram_tensor("wvh", [128, NDC, HD], F8, kind="ExternalInput")
    wvl = nc.dram_tensor("wvl", [128, NDC, HD], F8, kind="ExternalInput")
    woh = nc.dram_tensor("woh", [128, HPC, D], F8, kind="ExternalInput")
    wol = nc.dram_tensor("wol", [128, HPC, D], F8, kind="ExternalInput")
    cosT = nc.dram_tensor("cosT", [HD, S], BF, kind="ExternalInput")
    sinT = nc.dram_tensor("sinT", [HD, S], BF, kind="ExternalInput")
    rmat = nc.dram_tensor("rmat", [HD, HD], BF, kind="ExternalInput")
    masks = nc.dram_tensor("masks", [128, 512], BF, kind="ExternalInput")
    out = nc.dram_tensor("out", [S, D], BF, kind="ExternalOutput")

    with tile.TileContext(nc) as tc:
        _emit(tc, nc, xh, xl, wqh, wql, wkh, wkl, wvh, wvl, woh, wol,
              cosT, sinT, rmat, masks, out)
    nc.compile()
    return nc


def _emit(tc, nc, xh, xl, wqh, wql, wkh, wkl, wvh, wvl, woh, wol,
          cosT, sinT, rmat, masks, out):
    import contextlib
    ctx = contextlib.ExitStack()
    with ctx:
        const = ctx.enter_context(tc.tile_pool(name="const", bufs=1))
        work = ctx.enter_context(tc.tile_pool(name="work", bufs=1))
        tmp = ctx.enter_context(tc.tile_pool(name="tmp", bufs=4))
        pt_pool = ctx.enter_context(tc.tile_pool(name="pt", bufs=8))
        outp = ctx.enter_context(tc.tile_pool(name="outp", bufs=3))
        ps = ctx.enter_context(tc.tile_pool(name="ps", bufs=8, space="PSUM"))

        # ---- DMA emission, ordered to pace the chunk-major PE stream ----
        rmat_sb = const.tile([HD, HD], BF, tag="rmat")
        ones_sb = const.tile([128, 1], BF, tag="ones")
        nc.vector.memset(ones_sb[:], 1.0)
        ebias = const.tile([128, 1], F32, tag="ebias")
        nc.vector.memset(ebias[:], EXP_SHIFT)

        wk_sb = const.tile([128, 2, NDC, HD], F8, tag="wk")   # dim1: hi/lo
        nc.sync.dma_start(out=wk_sb[:, 0, :, :], in_=wkh.ap())
        nc.sync.dma_start(out=wk_sb[:, 1, :, :], in_=wkl.ap())
        nc.sync.dma_start(out=rmat_sb[:], in_=rmat.ap())

        gxh, gxl = {}, {}
        gqh, gql = {}, {}
        for c in range(0, NDC, 2):
            gxh[c] = const.tile([128, 2, S], F8, tag=f"xh{c//2}", name=f"xh{c//2}")
            gxl[c] = const.tile([128, 2, S], F8, tag=f"xl{c//2}", name=f"xl{c//2}")
        # half-0 of x plus all of wq, interleaved chunk-major
        for c in range(0, NDC, 2):
            if c % 4 == 0:
                g = const.tile([128, 4, QW], F8, tag=f"qh{c//4}", name=f"qh{c//4}")
                nc.sync.dma_start(out=g[:], in_=wqh.ap()[:, c:c + 4, :])
                gqh[c], gqh[c + 2] = g[:, 0:2, :], g[:, 2:4, :]
            nc.sync.dma_start(out=gxh[c][:, :, 0:512], in_=xh.ap()[:, c:c + 2, 0:512])
            if c % 4 == 2:
                g = const.tile([128, 4, QW], F8, tag=f"ql{c//4}", name=f"ql{c//4}")
                nc.sync.dma_start(out=g[:], in_=wql.ap()[:, c - 2:c + 2, :])
                gql[c - 2], gql[c] = g[:, 0:2, :], g[:, 2:4, :]
            nc.sync.dma_start(out=gxl[c][:, :, 0:512], in_=xl.ap()[:, c:c + 2, 0:512])
        cos_sb = const.tile([HD, S], BF, tag="cos")
        nc.sync.dma_start(out=cos_sb[:], in_=cosT.ap())
        sin_sb = const.tile([HD, S], BF, tag="sin")
        nc.sync.dma_start(out=sin_sb[:], in_=sinT.ap())
        # half-1 of x
        for c in range(0, NDC, 2):
            nc.sync.dma_start(out=gxh[c][:, :, 512:S], in_=xh.ap()[:, c:c + 2, 512:S])
            nc.sync.dma_start(out=gxl[c][:, :, 512:S], in_=xl.ap()[:, c:c + 2, 512:S])
        wv_sb = const.tile([128, 2, NDC, HD], F8, tag="wv")
        nc.sync.dma_start(out=wv_sb[:, 0, :, :], in_=wvh.ap())
        nc.sync.dma_start(out=wv_sb[:, 1, :, :], in_=wvl.ap())
        mask_sb = const.tile([128, 512], BF, tag="mask")
        nc.sync.dma_start(out=mask_sb[:], in_=masks.ap())
        wo_sb = const.tile([128, 2, HPC, D], F8, tag="wo")    # dim1: hi/lo
        for n in range(2):
            sl = slice(n * 2048, (n + 1) * 2048)
            nc.sync.dma_start(out=wo_sb[:, 0, :, sl], in_=woh.ap()[:, :, sl])
            nc.sync.dma_start(out=wo_sb[:, 1, :, sl], in_=wol.ap()[:, :, sl])

        # persistent activations
        khat = work.tile([HD, S], BF, tag="khat")
        qhat = [work.tile([HD, S], BF, tag=f"qhat{h}", name=f"qhat{h}")
                for h in range(HPC)]
        v_sb = [work.tile([128, HD], BF, tag=f"v{i}", name=f"v{i}")
                for i in range(8)]
        ctx_hi = [work.tile([128, 2, S], F8, tag=f"cth{u}", name=f"cth{u}")
                  for u in range(2)]
        ctx_lo = [work.tile([128, 2, S], F8, tag=f"ctl{u}", name=f"ctl{u}")
                  for u in range(2)]

        # ---- K+Q projections: chunk-major across 5 chains per s-quarter ----
        # Per chunk pair, all five tensors advance their 3-term DoubleRow
        # chains, so the PE stream follows the x/wq DMA arrival order.
        # RoPE for each finished s-half is queued and its PE/Act/DVE work is
        # injected into later quarters' streams (and the v-projection).
        TENS = [("k", khat, lambda c: wk_sb[:, 0, c:c + 2, :],
                 lambda c: wk_sb[:, 1, c:c + 2, :], 1.0 / (XS * WKS))]
        for h in range(HPC):
            hsl = slice(h * HD, (h + 1) * HD)
            TENS.append((f"q{h}", qhat[h],
                         lambda c, s=hsl: gqh[c][:, :, s],
                         lambda c, s=hsl: gql[c][:, :, s], 1.0 / (XS * WQS)))
        raws = {ti: work.tile([HD, S], BF, tag=f"raw{ti}", name=f"raw{ti}")
                for ti in range(5)}

        pend = []   # queued rope-finish closures (one per (tensor, half))

        def inject_rope():
            if pend:
                pend.pop(0)()

        def rope_half(ti, half):
            name, dst, _, _, _ = TENS[ti]
            sl = slice(half * 512, (half + 1) * 512)
            t1 = tmp.tile([HD, 512], BF, tag="rope_t1", name="rope_t1", bufs=2)
            nc.vector.tensor_mul(t1[:], raws[ti][:, sl], cos_sb[:, sl])
            rq = ps.tile([HD, 512], F32, tag="ps", name="rq")
            nc.tensor.matmul(rq[:], rmat_sb[:], raws[ti][:, sl],
                             start=True, stop=True)
            rqs = tmp.tile([HD, 512], BF, tag="rope_rqs", name="rope_rqs", bufs=2)
            nc.scalar.activation(rqs[:], rq[:],
                                 mybir.ActivationFunctionType.Copy)
            t2 = tmp.tile([HD, 512], BF, tag="rope_t2", name="rope_t2", bufs=2)
            nc.vector.tensor_mul(t2[:], rqs[:], sin_sb[:, sl])
            nc.vector.tensor_add(dst[:, sl], t1[:], t2[:])

        for half in (0, 1):
            for qr in (0, 1):
                quarter = 2 * half + qr
                ssl = slice(quarter * 256, (quarter + 1) * 256)
                chains = [ps.tile([128, 256], F32, tag="ps", name=f"ch{ti}")
                          for ti in range(5)]
                for p in range(NP):
                    c = 2 * p
                    for ti, (_, _, whi, wlo, _) in enumerate(TENS):
                        pp = chains[ti]
                        nc.tensor.matmul(pp[:], whi(c), gxh[c][:, :, ssl],
                                         start=(p == 0), stop=False,
                                         perf_mode=DR)
                        nc.tensor.matmul(pp[:], whi(c), gxl[c][:, :, ssl],
                                         start=False, stop=False, perf_mode=DR)
                        nc.tensor.matmul(pp[:], wlo(c), gxh[c][:, :, ssl],
                                         start=False, stop=(p == NP - 1),
                                         perf_mode=DR)
                    if p in (5, 11):
                        inject_rope()
                for ti, (_, _, _, _, descale) in enumerate(TENS):
                    nc.scalar.activation(raws[ti][:, ssl], chains[ti][:],
                                         mybir.ActivationFunctionType.Copy,
                                         scale=descale)
            for ti in range(5):
                pend.append(lambda t=ti, hf=half: rope_half(t, hf))

        # ---- V projection (x stationary, token-major out) ----
        for i in range(8):
            tsl = slice(i * 128, (i + 1) * 128)
            vp = ps.tile([128, HD], F32, tag="ps", name="vp")
            for p in range(NP):
                c = 2 * p
                nc.tensor.matmul(vp[:], gxh[c][:, :, tsl], wv_sb[:, 0, c:c + 2, :],
                                 start=(p == 0), stop=False, perf_mode=DR)
            for p in range(NP):
                c = 2 * p
                nc.tensor.matmul(vp[:], gxl[c][:, :, tsl], wv_sb[:, 0, c:c + 2, :],
                                 start=False, stop=False, perf_mode=DR)
            for p in range(NP):
                c = 2 * p
                nc.tensor.matmul(vp[:], gxh[c][:, :, tsl], wv_sb[:, 1, c:c + 2, :],
                                 start=False, stop=(p == NP - 1), perf_mode=DR)
            nc.scalar.activation(v_sb[i][:], vp[:],
                                 mybir.ActivationFunctionType.Copy,
                                 scale=1.0 / (XS * WVS))
            inject_rope()
        while pend:
            inject_rope()

        # ---- attention + out-proj, software-pipelined ----
        def emit_scores(h, b):
            ssl = slice(b * 256, (b + 1) * 256)
            pts = []
            for tp in range(b + 1):
                st = ps.tile([128, 512], F32, tag="ps", name="st")
                for i in range(2):
                    t0 = (2 * tp + i) * 128
                    nc.tensor.matmul(st[:, i * 256:(i + 1) * 256],
                                     khat[:, t0:t0 + 128], qhat[h][:, ssl],
                                     start=True, stop=True)
                pt = pt_pool.tile([128, 512], BF, tag="pt", name="pt")
                nc.scalar.activation(pt[:], st[:],
                                     mybir.ActivationFunctionType.Exp,
                                     bias=ebias[:])
                if tp == b:
                    nc.vector.tensor_mul(pt[:], pt[:], mask_sb[:])
                pts.append(pt)
            return pts

        def emit_denctx(h, b, pts):
            ssl = slice(b * 256, (b + 1) * 256)
            den = ps.tile([1, 256], F32, tag="ps", name="den")
            n_mm = 2 * (b + 1)
            k = 0
            for pt in pts:
                for i in range(2):
                    nc.tensor.matmul(den[:], ones_sb[:],
                                     pt[:, i * 256:(i + 1) * 256],
                                     start=(k == 0), stop=(k == n_mm - 1))
                    k += 1
            cx = ps.tile([HD, 256], F32, tag="ps", name="cx")
            k = 0
            for tp, pt in enumerate(pts):
                for i in range(2):
                    nc.tensor.matmul(cx[:], v_sb[2 * tp + i][:],
                                     pt[:, i * 256:(i + 1) * 256],
                                     start=(k == 0), stop=(k == n_mm - 1))
                    k += 1
            rec = tmp.tile([1, 256], F32, tag="rec", name="rec", bufs=2)
            nc.vector.reciprocal(rec[:], den[:])
            bc = tmp.tile([128, 256], F32, tag="bc", name="bc", bufs=2)
            nc.gpsimd.partition_broadcast(bc[:], rec[:])
            ctxn = tmp.tile([HD, 256], F32, tag="ctxn", name="ctxn", bufs=2)
            nc.vector.scalar_tensor_tensor(
                ctxn[:], cx[:], CTXS, bc[:],
                op0=mybir.AluOpType.mult, op1=mybir.AluOpType.mult)
            u, par = divmod(h, 2)
            nc.scalar.activation(ctx_hi[u][:, par, ssl], ctxn[:],
                                 mybir.ActivationFunctionType.Copy)
            nc.vector.tensor_sub(ctx_lo[u][:, par, ssl], ctxn[:],
                                 ctx_hi[u][:, par, ssl])

        descale = 1.0 / (CTXS * WOS)

        def emit_outproj(t8):
            tsl = slice(t8 * 128, (t8 + 1) * 128)
            for n4 in range(4):
                ot = outp.tile([128, 1024], BF, tag="ot", name="ot")
                for sub in range(4):
                    n = 4 * n4 + sub
                    nsl = slice(n * 256, (n + 1) * 256)
                    op = ps.tile([128, 256], F32, tag="ps", name="op")
                    k = 0
                    for u in range(2):
                        for chi, whi in ((ctx_hi, 0), (ctx_lo, 0), (ctx_hi, 1)):
                            nc.tensor.matmul(
                                op[:], chi[u][:, :, tsl],
                                wo_sb[:, whi, 2 * u:2 * u + 2, nsl],
                                start=(k == 0), stop=(k == 5), perf_mode=DR)
                            k += 1
                    eng = nc.vector if (sub % 2 == 0) else nc.gpsimd
                    eng.tensor_scalar_mul(ot[:, sub * 256:(sub + 1) * 256],
                                          op[:], descale)
                nc.sync.dma_start(
                    out=out.ap()[tsl, n4 * 1024:(n4 + 1) * 1024], in_=ot[:])

        stages = [(h, b) for b in range(4) for h in range(HPC)]
        prev = None
        outq = []
        for hb in stages:
            pts = emit_scores(*hb)
            if prev is not None:
                (ph, pb), ppts = prev
                emit_denctx(ph, pb, ppts)
                if ph == HPC - 1:
                    outq.extend([2 * pb, 2 * pb + 1])
            if outq:
                emit_outproj(outq.pop(0))
            prev = (hb, pts)
        (ph, pb), ppts = prev
        emit_denctx(ph, pb, ppts)
        outq.extend([2 * pb, 2 * pb + 1])
        for t8 in outq:
            emit_outproj(t8)


def _prep_inputs(x, cos, sin, Wq, Wk, Wv, Wo):
    """Host-side shard + hi/lo fp8 quantization. Returns per-core inputs."""
    bf = ml_dtypes.bfloat16
    f8 = ml_dtypes.float8_e4m3

    def hilo(a, s):
        hi = np.asarray(a * s, np.float32).astype(f8)
        lo = (np.asarray(a * s, np.float32) - hi.astype(np.float32)).astype(f8)
        return hi, lo

    x2 = np.asarray(x, np.float32).reshape(S, D)
    xTh = np.ascontiguousarray(x2.T).reshape(NDC, 128, S).transpose(1, 0, 2)
    xh_, xl_ = hilo(np.ascontiguousarray(xTh), XS)

    cosT = np.ascontiguousarray(np.asarray(cos, np.float32).T).astype(bf)
    sinT = np.ascontiguousarray(np.asarray(sin, np.float32).T).astype(bf)

    rmat = np.zeros((HD, HD), np.float32)
    half = HD // 2
    rmat[np.arange(half), np.arange(half) + half] = 1.0
    rmat[np.arange(half) + half, np.arange(half)] = -1.0
    rmat = rmat.astype(bf)

    # diagonal pair mask: keep when t_local (= i*128 + p) <= s_local
    lt = np.arange(128)[:, None]
    ls = np.arange(256)[None, :]
    masks = np.concatenate([(lt + 128 * i <= ls) for i in range(2)], axis=1)
    masks = np.ascontiguousarray(masks).astype(bf)     # [128, 512]

    scale = 1.0 / np.sqrt(np.float32(HD))
    Wq_ = np.asarray(Wq, np.float32) * scale
    Wk_ = np.asarray(Wk, np.float32)
    Wv_ = np.asarray(Wv, np.float32)
    Wo_ = np.asarray(Wo, np.float32)

    def chunked(w):  # [D, m] -> [128, NDC, m]
        m = w.shape[1]
        return np.ascontiguousarray(
            w.reshape(NDC, 128, m).transpose(1, 0, 2))

    in_maps = []
    for r in range(N_CORES):
        wqh_, wql_ = hilo(chunked(Wq_[:, r * QW:(r + 1) * QW]), WQS)
        wkh_, wkl_ = hilo(chunked(Wk_[:, r * HD:(r + 1) * HD]), WKS)
        wvh_, wvl_ = hilo(chunked(Wv_[:, r * HD:(r + 1) * HD]), WVS)
        wo_r = np.ascontiguousarray(
            Wo_[r * QW:(r + 1) * QW, :].reshape(HPC, 128, D)
            .transpose(1, 0, 2))
        woh_, wol_ = hilo(wo_r, WOS)
        in_maps.append({
            "xh": xh_, "xl": xl_, "wqh": wqh_, "wql": wql_,
            "wkh": wkh_, "wkl": wkl_, "wvh": wvh_, "wvl": wvl_,
            "woh": woh_, "wol": wol_,
            "cosT": cosT, "sinT": sinT, "rmat": rmat, "masks": masks,
        })
    return in_maps


def get_nc():
    if "nc" not in _CACHE:
        _CACHE["nc"] = _build()
    return _CACHE["nc"]


def kernel(x, mask, cos, sin, Wq, Wk, Wv, Wo):
    nc = get_nc()
    in_maps = _prep_inputs(x, cos, sin, Wq, Wk, Wv, Wo)
    res = run_bass_kernel_spmd(nc, in_maps, core_ids=list(range(N_CORES)))
    acc = np.zeros((S, D), np.float32)
    for r in range(N_CORES):
        acc += res.results[r]["out"].astype(np.float32)
    return acc[None]


if __name__ == "__main__":
    print("built:", get_nc() is not None)


# revision 9
# speedup vs baseline: 1.1799x; 1.0460x over previous
"""Grouped-query attention, tensor-parallel over heads across 8 TRN2 NeuronCores.

Problem (hardcoded): x[1,1024,4096] @ Wq/Wk/Wv -> RoPE -> causal GQA
(32 q heads, 8 kv groups, head_dim 128) -> out proj Wo -> [1,1024,4096].

Sharding: core r owns q heads 4r..4r+3 and kv group r (Wq/Wk/Wv column
shards, Wo row shard). Each core computes a full [1024,4096] partial of
the output projection; the host sums the 8 partials (the "all-reduce").

Device kernel (per core): the big GEMMs (Q/K/V projections, out-proj)
run in fp8e4 DoubleRow mode, which processes two 128-deep contraction
chunks per instruction at half the cycles/row of bf16.  Precision is
recovered with a 3-term hi/lo split quantization (x_hi@W_hi + x_lo@W_hi
+ x_hi@W_lo), where hi and lo shares one power-of-2 scale so all terms
accumulate in a single PSUM chain; measured end-to-end error matches
bf16.  The attention core (scores, exp, denominators, ctx) stays bf16
with 256-wide s-blocks and causal tile skipping.
"""

import numpy as np
import ml_dtypes

import concourse.bass as bass
import concourse.bacc as bacc
import concourse.mybir as mybir
import concourse.tile as tile
from concourse.bass_utils import run_bass_kernel_spmd

S = 1024          # sequence length
D = 4096          # model dim
H = 32            # query heads (global)
G = 8             # kv groups (global)
HD = 128          # head dim
N_CORES = 8
HPC = H // N_CORES   # 4 query heads per core
QW = HPC * HD        # 512 q-proj cols per core
NDC = D // 128       # 32 contraction chunks
NP = NDC // 2        # 16 DoubleRow chunk pairs
BF = mybir.dt.bfloat16
F8 = mybir.dt.float8e4
F32 = mybir.dt.float32
DR = mybir.MatmulPerfMode.DoubleRow

# quantization scales (powers of 2; hi and lo share the scale so every
# 3-term matmul accumulates in one PSUM chain)
XS = 16.0
WQS = 8192.0        # applied to Wq/sqrt(HD)
WKS = 1024.0
WVS = 1024.0
WOS = 1024.0
CTXS = 16.0
EXP_SHIFT = -6.0    # exp(s - 6): keeps bf16 P comfortably in range

_CACHE = {}


def _build():
    nc = bacc.Bacc("TRN2", target_bir_lowering=False, debug=False,
                   num_devices=N_CORES)

    xh = nc.dram_tensor("xh", [128, NDC, S], F8, kind="ExternalInput")
    xl = nc.dram_tensor("xl", [128, NDC, S], F8, kind="ExternalInput")
    wqh = nc.dram_tensor("wqh", [128, NDC, QW], F8, kind="ExternalInput")
    wql = nc.dram_tensor("wql", [128, NDC, QW], F8, kind="ExternalInput")
    wkh = nc.dram_tensor("wkh", [128, NDC, HD], F8, kind="ExternalInput")
    wkl = nc.dram_tensor("wkl", [128, NDC, HD], F8, kind="ExternalInput")
    wvh = nc.dram_tensor("wvh", [128, NDC, HD], F8, kind="ExternalInput")
    wvl = nc.dram_tensor("wvl", [128, NDC, HD], F8, kind="ExternalInput")
    woh = nc.dram_tensor("woh", [128, HPC, D], F8, kind="ExternalInput")
    wol = nc.dram_tensor("wol", [128, HPC, D], F8, kind="ExternalInput")
    cosT = nc.dram_tensor("cosT", [HD, S], BF, kind="ExternalInput")
    sinT = nc.dram_tensor("sinT", [HD, S], BF, kind="ExternalInput")
    rmat = nc.dram_tensor("rmat", [HD, HD], BF, kind="ExternalInput")
    masks = nc.dram_tensor("masks", [128, 512], BF, kind="ExternalInput")
    out = nc.dram_tensor("out", [S, D], BF, kind="ExternalOutput")

    with tile.TileContext(nc) as tc:
        _emit(tc, nc, xh, xl, wqh, wql, wkh, wkl, wvh, wvl, woh, wol,
              cosT, sinT, rmat, masks, out)
    nc.compile()
    return nc


def _emit(tc, nc, xh, xl, wqh, wql, wkh, wkl, wvh, wvl, woh, wol,
          cosT, sinT, rmat, masks, out):
    import contextlib
    ctx = contextlib.ExitStack()
    with ctx:
        const = ctx.enter_context(tc.tile_pool(name="const", bufs=1))
        work = ctx.enter_context(tc.tile_pool(name="work", bufs=1))
        tmp = ctx.enter_context(tc.tile_pool(name="tmp", bufs=4))
        pt_pool = ctx.enter_context(tc.tile_pool(name="pt", bufs=8))
        outp = ctx.enter_context(tc.tile_pool(name="outp", bufs=3))
        ps = ctx.enter_context(tc.tile_pool(name="ps", bufs=8, space="PSUM"))

        # ---- DMA emission, ordered to pace the chunk-major PE stream ----
        rmat_sb = const.tile([HD, HD], BF, tag="rmat")
        ones_sb = const.tile([128, 1], BF, tag="ones")
        nc.vector.memset(ones_sb[:], 1.0)
        ebias = const.tile([128, 1], F32, tag="ebias")
        nc.vector.memset(ebias[:], EXP_SHIFT)

        wk_sb = const.tile([128, 2, NDC, HD], F8, tag="wk")   # dim1: hi/lo
        nc.sync.dma_start(out=wk_sb[:, 0, :, :], in_=wkh.ap())
        nc.sync.dma_start(out=wk_sb[:, 1, :, :], in_=wkl.ap())
        nc.sync.dma_start(out=rmat_sb[:], in_=rmat.ap())

        gxh, gxl = {}, {}
        gqh, gql = {}, {}
        for c in range(0, NDC, 2):
            gxh[c] = const.tile([128, 2, S], F8, tag=f"xh{c//2}", name=f"xh{c//2}")
            gxl[c] = const.tile([128, 2, S], F8, tag=f"xl{c//2}", name=f"xl{c//2}")
        # half-0 of x plus all of wq, interleaved chunk-major
        for c in range(0, NDC, 2):
            if c % 4 == 0:
                g = const.tile([128, 4, QW], F8, tag=f"qh{c//4}", name=f"qh{c//4}")
                nc.sync.dma_start(out=g[:], in_=wqh.ap()[:, c:c + 4, :])
                gqh[c], gqh[c + 2] = g[:, 0:2, :], g[:, 2:4, :]
            nc.sync.dma_start(out=gxh[c][:, :, 0:512], in_=xh.ap()[:, c:c + 2, 0:512])
            if c % 4 == 2:
                g = const.tile([128, 4, QW], F8, tag=f"ql{c//4}", name=f"ql{c//4}")
                nc.sync.dma_start(out=g[:], in_=wql.ap()[:, c - 2:c + 2, :])
                gql[c - 2], gql[c] = g[:, 0:2, :], g[:, 2:4, :]
            nc.sync.dma_start(out=gxl[c][:, :, 0:512], in_=xl.ap()[:, c:c + 2, 0:512])
        cos_sb = const.tile([HD, S], BF, tag="cos")
        nc.sync.dma_start(out=cos_sb[:], in_=cosT.ap())
        sin_sb = const.tile([HD, S], BF, tag="sin")
        nc.sync.dma_start(out=sin_sb[:], in_=sinT.ap())
        # half-1 of x
        for c in range(0, NDC, 2):
            nc.sync.dma_start(out=gxh[c][:, :, 512:S], in_=xh.ap()[:, c:c + 2, 512:S])
            nc.sync.dma_start(out=gxl[c][:, :, 512:S], in_=xl.ap()[:, c:c + 2, 512:S])
        wv_sb = const.tile([128, 2, NDC, HD], F8, tag="wv")
        nc.sync.dma_start(out=wv_sb[:, 0, :, :], in_=wvh.ap())
        nc.sync.dma_start(out=wv_sb[:, 1, :, :], in_=wvl.ap())
        mask_sb = const.tile([128, 512], BF, tag="mask")
        nc.sync.dma_start(out=mask_sb[:], in_=masks.ap())
        wo_sb = const.tile([128, 2, HPC, D], F8, tag="wo")    # dim1: hi/lo
        for n in range(2):
            sl = slice(n * 2048, (n + 1) * 2048)
            nc.sync.dma_start(out=wo_sb[:, 0, :, sl], in_=woh.ap()[:, :, sl])
            nc.sync.dma_start(out=wo_sb[:, 1, :, sl], in_=wol.ap()[:, :, sl])

        # persistent activations
        khat = work.tile([HD, S], BF, tag="khat")
        qhat = [work.tile([HD, S], BF, tag=f"qhat{h}", name=f"qhat{h}")
                for h in range(HPC)]
        v_sb = [work.tile([128, HD], BF, tag=f"v{i}", name=f"v{i}")
                for i in range(8)]
        ctx_hi = [work.tile([128, 2, S], F8, tag=f"cth{u}", name=f"cth{u}")
                  for u in range(2)]
        ctx_lo = [work.tile([128, 2, S], F8, tag=f"ctl{u}", name=f"ctl{u}")
                  for u in range(2)]

        # ---- K+Q projections: chunk-major across 5 chains per s-quarter ----
        # Per chunk pair, all five tensors advance their 3-term DoubleRow
        # chains, so the PE stream follows the x/wq DMA arrival order.
        # RoPE for each finished s-half is queued and its PE/Act/DVE work is
        # injected into later quarters' streams (and the v-projection).
        TENS = [("k", khat, lambda c: wk_sb[:, 0, c:c + 2, :],
                 lambda c: wk_sb[:, 1, c:c + 2, :], 1.0 / (XS * WKS))]
        for h in range(HPC):
            hsl = slice(h * HD, (h + 1) * HD)
            TENS.append((f"q{h}", qhat[h],
                         lambda c, s=hsl: gqh[c][:, :, s],
                         lambda c, s=hsl: gql[c][:, :, s], 1.0 / (XS * WQS)))
        raws = {ti: work.tile([HD, S], BF, tag=f"raw{ti}", name=f"raw{ti}")
                for ti in range(5)}

        pend = []   # queued rope-finish closures (one per (tensor, half))

        def inject_rope():
            if pend:
                pend.pop(0)()

        def rope_half(ti, half):
            name, dst, _, _, _ = TENS[ti]
            sl = slice(half * 512, (half + 1) * 512)
            t1 = tmp.tile([HD, 512], BF, tag="rope_t1", name="rope_t1", bufs=2)
            nc.vector.tensor_mul(t1[:], raws[ti][:, sl], cos_sb[:, sl])
            rq = ps.tile([HD, 512], F32, tag="ps", name="rq")
            nc.tensor.matmul(rq[:], rmat_sb[:], raws[ti][:, sl],
                             start=True, stop=True)
            rqs = tmp.tile([HD, 512], BF, tag="rope_rqs", name="rope_rqs", bufs=2)
            nc.scalar.activation(rqs[:], rq[:],
                                 mybir.ActivationFunctionType.Copy)
            t2 = tmp.tile([HD, 512], BF, tag="rope_t2", name="rope_t2", bufs=2)
            nc.vector.tensor_mul(t2[:], rqs[:], sin_sb[:, sl])
            nc.vector.tensor_add(dst[:, sl], t1[:], t2[:])

        for half in (0, 1):
            hoff = half * 512
            chains = [ps.tile([128, 512], F32, tag="ps", name=f"ch{ti}")
                      for ti in range(5)]
            for p in range(NP):
                c = 2 * p
                for ti, (_, _, whi, wlo, _) in enumerate(TENS):
                    pp = chains[ti]
                    for q in range(2):
                        ssl = slice(hoff + q * 256, hoff + (q + 1) * 256)
                        osl = slice(q * 256, (q + 1) * 256)
                        nc.tensor.matmul(pp[:, osl], whi(c), gxh[c][:, :, ssl],
                                         start=(p == 0 and q == 0), stop=False,
                                         perf_mode=DR, skip_group_check=True)
                        nc.tensor.matmul(pp[:, osl], whi(c), gxl[c][:, :, ssl],
                                         start=False, stop=False,
                                         perf_mode=DR, skip_group_check=True)
                        nc.tensor.matmul(pp[:, osl], wlo(c), gxh[c][:, :, ssl],
                                         start=False,
                                         stop=(p == NP - 1 and q == 1),
                                         perf_mode=DR, skip_group_check=True)
                if p in (4, 9, 14):
                    inject_rope()
            for ti, (_, _, _, _, descale) in enumerate(TENS):
                nc.scalar.activation(raws[ti][:, hoff:hoff + 512], chains[ti][:],
                                     mybir.ActivationFunctionType.Copy,
                                     scale=descale)
            for ti in range(5):
                pend.append(lambda t=ti, hf=half: rope_half(t, hf))

        # ---- V projection (x stationary, token-major out) ----
        for i in range(8):
            tsl = slice(i * 128, (i + 1) * 128)
            vp = ps.tile([128, HD], F32, tag="ps", name="vp")
            for p in range(NP):
                c = 2 * p
                nc.tensor.matmul(vp[:], gxh[c][:, :, tsl], wv_sb[:, 0, c:c + 2, :],
                                 start=(p == 0), stop=False, perf_mode=DR)
            for p in range(NP):
                c = 2 * p
                nc.tensor.matmul(vp[:], gxl[c][:, :, tsl], wv_sb[:, 0, c:c + 2, :],
                                 start=False, stop=False, perf_mode=DR)
            for p in range(NP):
                c = 2 * p
                nc.tensor.matmul(vp[:], gxh[c][:, :, tsl], wv_sb[:, 1, c:c + 2, :],
                                 start=False, stop=(p == NP - 1), perf_mode=DR)
            nc.scalar.activation(v_sb[i][:], vp[:],
                                 mybir.ActivationFunctionType.Copy,
                                 scale=1.0 / (XS * WVS))
            inject_rope()
        while pend:
            inject_rope()

        # ---- attention + out-proj, software-pipelined ----
        def emit_scores(h, b):
            ssl = slice(b * 256, (b + 1) * 256)
            pts = []
            for tp in range(b + 1):
                st = ps.tile([128, 512], F32, tag="ps", name="st")
                for i in range(2):
                    t0 = (2 * tp + i) * 128
                    nc.tensor.matmul(st[:, i * 256:(i + 1) * 256],
                                     khat[:, t0:t0 + 128], qhat[h][:, ssl],
                                     start=True, stop=True)
                pt = pt_pool.tile([128, 512], BF, tag="pt", name="pt")
                nc.scalar.activation(pt[:], st[:],
                                     mybir.ActivationFunctionType.Exp,
                                     bias=ebias[:])
                if tp == b:
                    nc.vector.tensor_mul(pt[:], pt[:], mask_sb[:])
                pts.append(pt)
            return pts

        def emit_denctx(h, b, pts):
            ssl = slice(b * 256, (b + 1) * 256)
            den = ps.tile([1, 256], F32, tag="ps", name="den")
            n_mm = 2 * (b + 1)
            k = 0
            for pt in pts:
                for i in range(2):
                    nc.tensor.matmul(den[:], ones_sb[:],
                                     pt[:, i * 256:(i + 1) * 256],
                                     start=(k == 0), stop=(k == n_mm - 1))
                    k += 1
            cx = ps.tile([HD, 256], F32, tag="ps", name="cx")
            k = 0
            for tp, pt in enumerate(pts):
                for i in range(2):
                    nc.tensor.matmul(cx[:], v_sb[2 * tp + i][:],
                                     pt[:, i * 256:(i + 1) * 256],
                                     start=(k == 0), stop=(k == n_mm - 1))
                    k += 1
            rec = tmp.tile([1, 256], F32, tag="rec", name="rec", bufs=2)
            nc.vector.reciprocal(rec[:], den[:])
            bc = tmp.tile([128, 256], F32, tag="bc", name="bc", bufs=2)
            nc.gpsimd.partition_broadcast(bc[:], rec[:])
            ctxn = tmp.tile([HD, 256], F32, tag="ctxn", name="ctxn", bufs=2)
            nc.vector.scalar_tensor_tensor(
                ctxn[:], cx[:], CTXS, bc[:],
                op0=mybir.AluOpType.mult, op1=mybir.AluOpType.mult)
            u, par = divmod(h, 2)
            nc.scalar.activation(ctx_hi[u][:, par, ssl], ctxn[:],
                                 mybir.ActivationFunctionType.Copy)
            nc.vector.tensor_sub(ctx_lo[u][:, par, ssl], ctxn[:],
                                 ctx_hi[u][:, par, ssl])

        descale = 1.0 / (CTXS * WOS)

        def emit_outproj(t8):
            tsl = slice(t8 * 128, (t8 + 1) * 128)
            for n4 in range(4):
                ot = outp.tile([128, 1024], BF, tag="ot", name="ot")
                for sub in range(4):
                    n = 4 * n4 + sub
                    nsl = slice(n * 256, (n + 1) * 256)
                    op = ps.tile([128, 256], F32, tag="ps", name="op")
                    k = 0
                    for u in range(2):
                        for chi, whi in ((ctx_hi, 0), (ctx_lo, 0), (ctx_hi, 1)):
                            nc.tensor.matmul(
                                op[:], chi[u][:, :, tsl],
                                wo_sb[:, whi, 2 * u:2 * u + 2, nsl],
                                start=(k == 0), stop=(k == 5), perf_mode=DR)
                            k += 1
                    eng = nc.vector if (sub % 2 == 0) else nc.gpsimd
                    eng.tensor_scalar_mul(ot[:, sub * 256:(sub + 1) * 256],
                                          op[:], descale)
                nc.sync.dma_start(
                    out=out.ap()[tsl, n4 * 1024:(n4 + 1) * 1024], in_=ot[:])

        stages = [(h, b) for b in range(4) for h in range(HPC)]
        prev = None
        outq = []
        for hb in stages:
            pts = emit_scores(*hb)
            if prev is not None:
                (ph, pb), ppts = prev
                emit_denctx(ph, pb, ppts)
                if ph == HPC - 1:
                    outq.extend([2 * pb, 2 * pb + 1])
            if outq:
                emit_outproj(outq.pop(0))
            prev = (hb, pts)
        (ph, pb), ppts = prev
        emit_denctx(ph, pb, ppts)
        outq.extend([2 * pb, 2 * pb + 1])
        for t8 in outq:
            emit_outproj(t8)


def _prep_inputs(x, cos, sin, Wq, Wk, Wv, Wo):
    """Host-side shard + hi/lo fp8 quantization. Returns per-core inputs."""
    bf = ml_dtypes.bfloat16
    f8 = ml_dtypes.float8_e4m3

    def hilo(a, s):
        hi = np.asarray(a * s, np.float32).astype(f8)
        lo = (np.asarray(a * s, np.float32) - hi.astype(np.float32)).astype(f8)
        return hi, lo

    x2 = np.asarray(x, np.float32).reshape(S, D)
    xTh = np.ascontiguousarray(x2.T).reshape(NDC, 128, S).transpose(1, 0, 2)
    xh_, xl_ = hilo(np.ascontiguousarray(xTh), XS)

    cosT = np.ascontiguousarray(np.asarray(cos, np.float32).T).astype(bf)
    sinT = np.ascontiguousarray(np.asarray(sin, np.float32).T).astype(bf)

    rmat = np.zeros((HD, HD), np.float32)
    half = HD // 2
    rmat[np.arange(half), np.arange(half) + half] = 1.0
    rmat[np.arange(half) + half, np.arange(half)] = -1.0
    rmat = rmat.astype(bf)

    # diagonal pair mask: keep when t_local (= i*128 + p) <= s_local
    lt = np.arange(128)[:, None]
    ls = np.arange(256)[None, :]
    masks = np.concatenate([(lt + 128 * i <= ls) for i in range(2)], axis=1)
    masks = np.ascontiguousarray(masks).astype(bf)     # [128, 512]

    scale = 1.0 / np.sqrt(np.float32(HD))
    Wq_ = np.asarray(Wq, np.float32) * scale
    Wk_ = np.asarray(Wk, np.float32)
    Wv_ = np.asarray(Wv, np.float32)
    Wo_ = np.asarray(Wo, np.float32)

    def chunked(w):  # [D, m] -> [128, NDC, m]
        m = w.shape[1]
        return np.ascontiguousarray(
            w.reshape(NDC, 128, m).transpose(1, 0, 2))

    in_maps = []
    for r in range(N_CORES):
        wqh_, wql_ = hilo(chunked(Wq_[:, r * QW:(r + 1) * QW]), WQS)
        wkh_, wkl_ = hilo(chunked(Wk_[:, r * HD:(r + 1) * HD]), WKS)
        wvh_, wvl_ = hilo(chunked(Wv_[:, r * HD:(r + 1) * HD]), WVS)
        wo_r = np.ascontiguousarray(
            Wo_[r * QW:(r + 1) * QW, :].reshape(HPC, 128, D)
            .transpose(1, 0, 2))
        woh_, wol_ = hilo(wo_r, WOS)
        in_maps.append({
            "xh": xh_, "xl": xl_, "wqh": wqh_, "wql": wql_,
            "wkh": wkh_, "wkl": wkl_, "wvh": wvh_, "wvl": wvl_,
            "woh": woh_, "wol": wol_,
            "cosT": cosT, "sinT": sinT, "rmat": rmat, "masks": masks,
        })
    return in_maps


def get_nc():
    if "nc" not in _CACHE:
        _CACHE["nc"] = _build()
    return _CACHE["nc"]


def kernel(x, mask, cos, sin, Wq, Wk, Wv, Wo):
    nc = get_nc()
    in_maps = _prep_inputs(x, cos, sin, Wq, Wk, Wv, Wo)
    res = run_bass_kernel_spmd(nc, in_maps, core_ids=list(range(N_CORES)))
    acc = np.zeros((S, D), np.float32)
    for r in range(N_CORES):
        acc += res.results[r]["out"].astype(np.float32)
    return acc[None]


if __name__ == "__main__":
    print("built:", get_nc() is not None)


# revision 10
# speedup vs baseline: 1.1817x; 1.0014x over previous
"""Grouped-query attention, tensor-parallel over heads across 8 TRN2 NeuronCores.

Problem (hardcoded): x[1,1024,4096] @ Wq/Wk/Wv -> RoPE -> causal GQA
(32 q heads, 8 kv groups, head_dim 128) -> out proj Wo -> [1,1024,4096].

Sharding: core r owns q heads 4r..4r+3 and kv group r (Wq/Wk/Wv column
shards, Wo row shard). Each core computes a full [1024,4096] partial of
the output projection; the host sums the 8 partials (the "all-reduce").

Device kernel (per core): the big GEMMs (Q/K/V projections, out-proj)
run in fp8e4 DoubleRow mode, which processes two 128-deep contraction
chunks per instruction at half the cycles/row of bf16.  Precision is
recovered with a 3-term hi/lo split quantization (x_hi@W_hi + x_lo@W_hi
+ x_hi@W_lo), where hi and lo shares one power-of-2 scale so all terms
accumulate in a single PSUM chain; measured end-to-end error matches
bf16.  The attention core (scores, exp, denominators, ctx) stays bf16
with 256-wide s-blocks and causal tile skipping.
"""

import numpy as np
import ml_dtypes

import concourse.bass as bass
import concourse.bacc as bacc
import concourse.mybir as mybir
import concourse.tile as tile
from concourse.bass_utils import run_bass_kernel_spmd

S = 1024          # sequence length
D = 4096          # model dim
H = 32            # query heads (global)
G = 8             # kv groups (global)
HD = 128          # head dim
N_CORES = 8
HPC = H // N_CORES   # 4 query heads per core
QW = HPC * HD        # 512 q-proj cols per core
NDC = D // 128       # 32 contraction chunks
NP = NDC // 2        # 16 DoubleRow chunk pairs
BF = mybir.dt.bfloat16
F8 = mybir.dt.float8e4
F32 = mybir.dt.float32
DR = mybir.MatmulPerfMode.DoubleRow

# quantization scales (powers of 2; hi and lo share the scale so every
# 3-term matmul accumulates in one PSUM chain)
XS = 16.0
WQS = 8192.0        # applied to Wq/sqrt(HD)
WKS = 1024.0
WVS = 1024.0
WOS = 1024.0
CTXS = 16.0
EXP_SHIFT = -6.0    # exp(s - 6): keeps bf16 P comfortably in range

_CACHE = {}


def _build():
    nc = bacc.Bacc("TRN2", target_bir_lowering=False, debug=False,
                   num_devices=N_CORES)

    xh = nc.dram_tensor("xh", [128, NDC, S], F8, kind="ExternalInput")
    xl = nc.dram_tensor("xl", [128, NDC, S], F8, kind="ExternalInput")
    wqh = nc.dram_tensor("wqh", [128, NDC, QW], F8, kind="ExternalInput")
    wql = nc.dram_tensor("wql", [128, NDC, QW], F8, kind="ExternalInput")
    wkh = nc.dram_tensor("wkh", [128, NDC, HD], F8, kind="ExternalInput")
    wkl = nc.dram_tensor("wkl", [128, NDC, HD], F8, kind="ExternalInput")
    wvh = nc.dram_tensor("wvh", [128, NDC, HD], F8, kind="ExternalInput")
    wvl = nc.dram_tensor("wvl", [128, NDC, HD], F8, kind="ExternalInput")
    woh = nc.dram_tensor("woh", [128, HPC, D], F8, kind="ExternalInput")
    wol = nc.dram_tensor("wol", [128, HPC, D], F8, kind="ExternalInput")
    cosT = nc.dram_tensor("cosT", [HD, S], BF, kind="ExternalInput")
    sinT = nc.dram_tensor("sinT", [HD, S], BF, kind="ExternalInput")
    rmat = nc.dram_tensor("rmat", [HD, HD], BF, kind="ExternalInput")
    masks = nc.dram_tensor("masks", [128, 512], BF, kind="ExternalInput")
    out = nc.dram_tensor("out", [S, D], BF, kind="ExternalOutput")

    with tile.TileContext(nc) as tc:
        _emit(tc, nc, xh, xl, wqh, wql, wkh, wkl, wvh, wvl, woh, wol,
              cosT, sinT, rmat, masks, out)
    nc.compile()
    return nc


def _emit(tc, nc, xh, xl, wqh, wql, wkh, wkl, wvh, wvl, woh, wol,
          cosT, sinT, rmat, masks, out):
    import contextlib
    ctx = contextlib.ExitStack()
    with ctx:
        const = ctx.enter_context(tc.tile_pool(name="const", bufs=1))
        work = ctx.enter_context(tc.tile_pool(name="work", bufs=1))
        tmp = ctx.enter_context(tc.tile_pool(name="tmp", bufs=4))
        pt_pool = ctx.enter_context(tc.tile_pool(name="pt", bufs=8))
        outp = ctx.enter_context(tc.tile_pool(name="outp", bufs=3))
        ps = ctx.enter_context(tc.tile_pool(name="ps", bufs=8, space="PSUM"))

        # ---- DMA emission, ordered to pace the chunk-major PE stream ----
        rmat_sb = const.tile([HD, HD], BF, tag="rmat")
        ones_sb = const.tile([128, 1], BF, tag="ones")
        nc.vector.memset(ones_sb[:], 1.0)
        ebias = const.tile([128, 1], F32, tag="ebias")
        nc.vector.memset(ebias[:], EXP_SHIFT)

        wk_sb = const.tile([128, 2, NDC, HD], F8, tag="wk")   # dim1: hi/lo
        nc.sync.dma_start(out=rmat_sb[:], in_=rmat.ap())

        gx4h, gx4l = {}, {}
        gqh, gql = {}, {}
        for c in range(0, NDC, 4):
            gx4h[c] = const.tile([128, 4, S], F8, tag=f"xh{c//4}", name=f"xh{c//4}")
            gx4l[c] = const.tile([128, 4, S], F8, tag=f"xl{c//4}", name=f"xl{c//4}")
        gxh = {c: gx4h[c - c % 4][:, c % 4:c % 4 + 2, :] for c in range(0, NDC, 2)}
        gxl = {c: gx4l[c - c % 4][:, c % 4:c % 4 + 2, :] for c in range(0, NDC, 2)}
        # half-0 of x, wq, and wk interleaved in consumption order
        for c in range(0, NDC, 4):
            nc.sync.dma_start(out=wk_sb[:, 0, c:c + 4, :], in_=wkh.ap()[:, c:c + 4, :])
            nc.sync.dma_start(out=wk_sb[:, 1, c:c + 4, :], in_=wkl.ap()[:, c:c + 4, :])
            g = const.tile([128, 4, QW], F8, tag=f"qh{c//4}", name=f"qh{c//4}")
            nc.sync.dma_start(out=g[:], in_=wqh.ap()[:, c:c + 4, :])
            gqh[c], gqh[c + 2] = g[:, 0:2, :], g[:, 2:4, :]
            nc.sync.dma_start(out=gx4h[c][:, :, 0:512], in_=xh.ap()[:, c:c + 4, 0:512])
            g = const.tile([128, 4, QW], F8, tag=f"ql{c//4}", name=f"ql{c//4}")
            nc.sync.dma_start(out=g[:], in_=wql.ap()[:, c:c + 4, :])
            gql[c], gql[c + 2] = g[:, 0:2, :], g[:, 2:4, :]
            nc.sync.dma_start(out=gx4l[c][:, :, 0:512], in_=xl.ap()[:, c:c + 4, 0:512])
        cos_sb = const.tile([HD, S], BF, tag="cos")
        nc.sync.dma_start(out=cos_sb[:], in_=cosT.ap())
        sin_sb = const.tile([HD, S], BF, tag="sin")
        nc.sync.dma_start(out=sin_sb[:], in_=sinT.ap())
        # half-1 of x
        for c in range(0, NDC, 4):
            nc.sync.dma_start(out=gx4h[c][:, :, 512:S], in_=xh.ap()[:, c:c + 4, 512:S])
            nc.sync.dma_start(out=gx4l[c][:, :, 512:S], in_=xl.ap()[:, c:c + 4, 512:S])
        wv_sb = const.tile([128, 2, NDC, HD], F8, tag="wv")
        nc.sync.dma_start(out=wv_sb[:, 0, :, :], in_=wvh.ap())
        nc.sync.dma_start(out=wv_sb[:, 1, :, :], in_=wvl.ap())
        mask_sb = const.tile([128, 512], BF, tag="mask")
        nc.sync.dma_start(out=mask_sb[:], in_=masks.ap())
        wo_sb = const.tile([128, 2, HPC, D], F8, tag="wo")    # dim1: hi/lo
        for n in range(2):
            sl = slice(n * 2048, (n + 1) * 2048)
            nc.sync.dma_start(out=wo_sb[:, 0, :, sl], in_=woh.ap()[:, :, sl])
            nc.sync.dma_start(out=wo_sb[:, 1, :, sl], in_=wol.ap()[:, :, sl])

        # persistent activations
        khat = work.tile([HD, S], BF, tag="khat")
        qhat = [work.tile([HD, S], BF, tag=f"qhat{h}", name=f"qhat{h}")
                for h in range(HPC)]
        v_sb = [work.tile([128, HD], BF, tag=f"v{i}", name=f"v{i}")
                for i in range(8)]
        ctx_hi = [work.tile([128, 2, S], F8, tag=f"cth{u}", name=f"cth{u}")
                  for u in range(2)]
        ctx_lo = [work.tile([128, 2, S], F8, tag=f"ctl{u}", name=f"ctl{u}")
                  for u in range(2)]

        # ---- K+Q projections: chunk-major across 5 chains per s-quarter ----
        # Per chunk pair, all five tensors advance their 3-term DoubleRow
        # chains, so the PE stream follows the x/wq DMA arrival order.
        # RoPE for each finished s-half is queued and its PE/Act/DVE work is
        # injected into later quarters' streams (and the v-projection).
        TENS = [("k", khat, lambda c: wk_sb[:, 0, c:c + 2, :],
                 lambda c: wk_sb[:, 1, c:c + 2, :], 1.0 / (XS * WKS))]
        for h in range(HPC):
            hsl = slice(h * HD, (h + 1) * HD)
            TENS.append((f"q{h}", qhat[h],
                         lambda c, s=hsl: gqh[c][:, :, s],
                         lambda c, s=hsl: gql[c][:, :, s], 1.0 / (XS * WQS)))
        raws = {ti: work.tile([HD, S], BF, tag=f"raw{ti}", name=f"raw{ti}")
                for ti in range(5)}

        pend = []   # queued rope-finish closures (one per (tensor, half))

        def inject_rope():
            if pend:
                pend.pop(0)()

        def rope_half(ti, half):
            name, dst, _, _, _ = TENS[ti]
            sl = slice(half * 512, (half + 1) * 512)
            t1 = tmp.tile([HD, 512], BF, tag="rope_t1", name="rope_t1", bufs=2)
            nc.vector.tensor_mul(t1[:], raws[ti][:, sl], cos_sb[:, sl])
            rq = ps.tile([HD, 512], F32, tag="ps", name="rq")
            nc.tensor.matmul(rq[:], rmat_sb[:], raws[ti][:, sl],
                             start=True, stop=True)
            rqs = tmp.tile([HD, 512], BF, tag="rope_rqs", name="rope_rqs", bufs=2)
            nc.scalar.activation(rqs[:], rq[:],
                                 mybir.ActivationFunctionType.Copy)
            t2 = tmp.tile([HD, 512], BF, tag="rope_t2", name="rope_t2", bufs=2)
            nc.vector.tensor_mul(t2[:], rqs[:], sin_sb[:, sl])
            nc.vector.tensor_add(dst[:, sl], t1[:], t2[:])

        for half in (0, 1):
            hoff = half * 512
            chains = [ps.tile([128, 512], F32, tag="ps", name=f"ch{ti}")
                      for ti in range(5)]
            for p in range(NP):
                c = 2 * p
                for ti, (_, _, whi, wlo, _) in enumerate(TENS):
                    pp = chains[ti]
                    for q in range(2):
                        ssl = slice(hoff + q * 256, hoff + (q + 1) * 256)
                        osl = slice(q * 256, (q + 1) * 256)
                        nc.tensor.matmul(pp[:, osl], whi(c), gxh[c][:, :, ssl],
                                         start=(p == 0 and q == 0), stop=False,
                                         perf_mode=DR, skip_group_check=True)
                        nc.tensor.matmul(pp[:, osl], whi(c), gxl[c][:, :, ssl],
                                         start=False, stop=False,
                                         perf_mode=DR, skip_group_check=True)
                        nc.tensor.matmul(pp[:, osl], wlo(c), gxh[c][:, :, ssl],
                                         start=False,
                                         stop=(p == NP - 1 and q == 1),
                                         perf_mode=DR, skip_group_check=True)
                if p in (4, 9, 14):
                    inject_rope()
            for ti, (_, _, _, _, descale) in enumerate(TENS):
                nc.scalar.activation(raws[ti][:, hoff:hoff + 512], chains[ti][:],
                                     mybir.ActivationFunctionType.Copy,
                                     scale=descale)
            for ti in range(5):
                pend.append(lambda t=ti, hf=half: rope_half(t, hf))

        # ---- V projection (x stationary, token-major out) ----
        for i in range(8):
            tsl = slice(i * 128, (i + 1) * 128)
            vp = ps.tile([128, HD], F32, tag="ps", name="vp")
            for p in range(NP):
                c = 2 * p
                nc.tensor.matmul(vp[:], gxh[c][:, :, tsl], wv_sb[:, 0, c:c + 2, :],
                                 start=(p == 0), stop=False, perf_mode=DR)
            for p in range(NP):
                c = 2 * p
                nc.tensor.matmul(vp[:], gxl[c][:, :, tsl], wv_sb[:, 0, c:c + 2, :],
                                 start=False, stop=False, perf_mode=DR)
            for p in range(NP):
                c = 2 * p
                nc.tensor.matmul(vp[:], gxh[c][:, :, tsl], wv_sb[:, 1, c:c + 2, :],
                                 start=False, stop=(p == NP - 1), perf_mode=DR)
            nc.scalar.activation(v_sb[i][:], vp[:],
                                 mybir.ActivationFunctionType.Copy,
                                 scale=1.0 / (XS * WVS))
            inject_rope()
        while pend:
            inject_rope()

        # ---- attention + out-proj, software-pipelined ----
        def emit_scores(h, b):
            ssl = slice(b * 256, (b + 1) * 256)
            pts = []
            for tp in range(b + 1):
                st = ps.tile([128, 512], F32, tag="ps", name="st")
                for i in range(2):
                    t0 = (2 * tp + i) * 128
                    nc.tensor.matmul(st[:, i * 256:(i + 1) * 256],
                                     khat[:, t0:t0 + 128], qhat[h][:, ssl],
                                     start=True, stop=True)
                pt = pt_pool.tile([128, 512], BF, tag="pt", name="pt")
                nc.scalar.activation(pt[:], st[:],
                                     mybir.ActivationFunctionType.Exp,
                                     bias=ebias[:])
                if tp == b:
                    nc.vector.tensor_mul(pt[:], pt[:], mask_sb[:])
                pts.append(pt)
            return pts

        def emit_denctx(h, b, pts):
            ssl = slice(b * 256, (b + 1) * 256)
            den = ps.tile([1, 256], F32, tag="ps", name="den")
            n_mm = 2 * (b + 1)
            k = 0
            for pt in pts:
                for i in range(2):
                    nc.tensor.matmul(den[:], ones_sb[:],
                                     pt[:, i * 256:(i + 1) * 256],
                                     start=(k == 0), stop=(k == n_mm - 1))
                    k += 1
            cx = ps.tile([HD, 256], F32, tag="ps", name="cx")
            k = 0
            for tp, pt in enumerate(pts):
                for i in range(2):
                    nc.tensor.matmul(cx[:], v_sb[2 * tp + i][:],
                                     pt[:, i * 256:(i + 1) * 256],
                                     start=(k == 0), stop=(k == n_mm - 1))
                    k += 1
            rec = tmp.tile([1, 256], F32, tag="rec", name="rec", bufs=2)
            nc.vector.reciprocal(rec[:], den[:])
            bc = tmp.tile([128, 256], F32, tag="bc", name="bc", bufs=2)
            nc.gpsimd.partition_broadcast(bc[:], rec[:])
            ctxn = tmp.tile([HD, 256], F32, tag="ctxn", name="ctxn", bufs=2)
            nc.vector.scalar_tensor_tensor(
                ctxn[:], cx[:], CTXS, bc[:],
                op0=mybir.AluOpType.mult, op1=mybir.AluOpType.mult)
            u, par = divmod(h, 2)
            nc.scalar.activation(ctx_hi[u][:, par, ssl], ctxn[:],
                                 mybir.ActivationFunctionType.Copy)
            nc.vector.tensor_sub(ctx_lo[u][:, par, ssl], ctxn[:],
                                 ctx_hi[u][:, par, ssl])

        descale = 1.0 / (CTXS * WOS)

        def emit_outproj(t8):
            tsl = slice(t8 * 128, (t8 + 1) * 128)
            for n4 in range(4):
                ot = outp.tile([128, 1024], BF, tag="ot", name="ot")
                for sub in range(4):
                    n = 4 * n4 + sub
                    nsl = slice(n * 256, (n + 1) * 256)
                    op = ps.tile([128, 256], F32, tag="ps", name="op")
                    k = 0
                    for u in range(2):
                        for chi, whi in ((ctx_hi, 0), (ctx_lo, 0), (ctx_hi, 1)):
                            nc.tensor.matmul(
                                op[:], chi[u][:, :, tsl],
                                wo_sb[:, whi, 2 * u:2 * u + 2, nsl],
                                start=(k == 0), stop=(k == 5), perf_mode=DR)
                            k += 1
                    eng = nc.vector if (sub % 2 == 0) else nc.gpsimd
                    eng.tensor_scalar_mul(ot[:, sub * 256:(sub + 1) * 256],
                                          op[:], descale)
                nc.sync.dma_start(
                    out=out.ap()[tsl, n4 * 1024:(n4 + 1) * 1024], in_=ot[:])

        stages = [(h, b) for b in range(4) for h in range(HPC)]
        prev = None
        outq = []
        for hb in stages:
            pts = emit_scores(*hb)
            if prev is not None:
                (ph, pb), ppts = prev
                emit_denctx(ph, pb, ppts)
                if ph == HPC - 1:
                    outq.extend([2 * pb, 2 * pb + 1])
            if outq:
                emit_outproj(outq.pop(0))
            prev = (hb, pts)
        (ph, pb), ppts = prev
        emit_denctx(ph, pb, ppts)
        outq.extend([2 * pb, 2 * pb + 1])
        for t8 in outq:
            emit_outproj(t8)


def _prep_inputs(x, cos, sin, Wq, Wk, Wv, Wo):
    """Host-side shard + hi/lo fp8 quantization. Returns per-core inputs."""
    bf = ml_dtypes.bfloat16
    f8 = ml_dtypes.float8_e4m3

    def hilo(a, s):
        hi = np.asarray(a * s, np.float32).astype(f8)
        lo = (np.asarray(a * s, np.float32) - hi.astype(np.float32)).astype(f8)
        return hi, lo

    x2 = np.asarray(x, np.float32).reshape(S, D)
    xTh = np.ascontiguousarray(x2.T).reshape(NDC, 128, S).transpose(1, 0, 2)
    xh_, xl_ = hilo(np.ascontiguousarray(xTh), XS)

    cosT = np.ascontiguousarray(np.asarray(cos, np.float32).T).astype(bf)
    sinT = np.ascontiguousarray(np.asarray(sin, np.float32).T).astype(bf)

    rmat = np.zeros((HD, HD), np.float32)
    half = HD // 2
    rmat[np.arange(half), np.arange(half) + half] = 1.0
    rmat[np.arange(half) + half, np.arange(half)] = -1.0
    rmat = rmat.astype(bf)

    # diagonal pair mask: keep when t_local (= i*128 + p) <= s_local
    lt = np.arange(128)[:, None]
    ls = np.arange(256)[None, :]
    masks = np.concatenate([(lt + 128 * i <= ls) for i in range(2)], axis=1)
    masks = np.ascontiguousarray(masks).astype(bf)     # [128, 512]

    scale = 1.0 / np.sqrt(np.float32(HD))
    Wq_ = np.asarray(Wq, np.float32) * scale
    Wk_ = np.asarray(Wk, np.float32)
    Wv_ = np.asarray(Wv, np.float32)
    Wo_ = np.asarray(Wo, np.float32)

    def chunked(w):  # [D, m] -> [128, NDC, m]
        m = w.shape[1]
        return np.ascontiguousarray(
            w.reshape(NDC, 128, m).transpose(1, 0, 2))

    in_maps = []
    for r in range(N_CORES):
        wqh_, wql_ = hilo(chunked(Wq_[:, r * QW:(r + 1) * QW]), WQS)
        wkh_, wkl_ = hilo(chunked(Wk_[:, r * HD:(r + 1) * HD]), WKS)
        wvh_, wvl_ = hilo(chunked(Wv_[:, r * HD:(r + 1) * HD]), WVS)
        wo_r = np.ascontiguousarray(
            Wo_[r * QW:(r + 1) * QW, :].reshape(HPC, 128, D)
            .transpose(1, 0, 2))
        woh_, wol_ = hilo(wo_r, WOS)
        in_maps.append({
            "xh": xh_, "xl": xl_, "wqh": wqh_, "wql": wql_,
            "wkh": wkh_, "wkl": wkl_, "wvh": wvh_, "wvl": wvl_,
            "woh": woh_, "wol": wol_,
            "cosT": cosT, "sinT": sinT, "rmat": rmat, "masks": masks,
        })
    return in_maps


def get_nc():
    if "nc" not in _CACHE:
        _CACHE["nc"] = _build()
    return _CACHE["nc"]


def kernel(x, mask, cos, sin, Wq, Wk, Wv, Wo):
    nc = get_nc()
    in_maps = _prep_inputs(x, cos, sin, Wq, Wk, Wv, Wo)
    res = run_bass_kernel_spmd(nc, in_maps, core_ids=list(range(N_CORES)))
    acc = np.zeros((S, D), np.float32)
    for r in range(N_CORES):
        acc += res.results[r]["out"].astype(np.float32)
    return acc[None]


if __name__ == "__main__":
    print("built:", get_nc() is not None)


# revision 11
# speedup vs baseline: 1.2034x; 1.0184x over previous
"""Grouped-query attention, tensor-parallel over heads across 8 TRN2 NeuronCores.

Problem (hardcoded): x[1,1024,4096] @ Wq/Wk/Wv -> RoPE -> causal GQA
(32 q heads, 8 kv groups, head_dim 128) -> out proj Wo -> [1,1024,4096].

Sharding: core r owns q heads 4r..4r+3 and kv group r (Wq/Wk/Wv column
shards, Wo row shard). Each core computes a full [1024,4096] partial of
the output projection; the host sums the 8 partials (the "all-reduce").

Device kernel (per core): the big GEMMs (Q/K/V projections, out-proj)
run in fp8e4 DoubleRow mode, which processes two 128-deep contraction
chunks per instruction at half the cycles/row of bf16.  Precision is
recovered with a 3-term hi/lo split quantization (x_hi@W_hi + x_lo@W_hi
+ x_hi@W_lo), where hi and lo shares one power-of-2 scale so all terms
accumulate in a single PSUM chain; measured end-to-end error matches
bf16.  The attention core (scores, exp, denominators, ctx) stays bf16
with 256-wide s-blocks and causal tile skipping.
"""

import numpy as np
import ml_dtypes

import concourse.bass as bass
import concourse.bacc as bacc
import concourse.mybir as mybir
import concourse.tile as tile
from concourse.bass_utils import run_bass_kernel_spmd

S = 1024          # sequence length
D = 4096          # model dim
H = 32            # query heads (global)
G = 8             # kv groups (global)
HD = 128          # head dim
N_CORES = 8
HPC = H // N_CORES   # 4 query heads per core
QW = HPC * HD        # 512 q-proj cols per core
NDC = D // 128       # 32 contraction chunks
NP = NDC // 2        # 16 DoubleRow chunk pairs
BF = mybir.dt.bfloat16
F8 = mybir.dt.float8e4
F32 = mybir.dt.float32
DR = mybir.MatmulPerfMode.DoubleRow

# quantization scales (powers of 2; hi and lo share the scale so every
# 3-term matmul accumulates in one PSUM chain)
XS = 16.0
WQS = 8192.0        # applied to Wq/sqrt(HD)
WKS = 1024.0
WVS = 1024.0
WOS = 1024.0
CTXS = 16.0
EXP_SHIFT = -6.0    # exp(s - 6): keeps bf16 P comfortably in range

_CACHE = {}


def _build():
    nc = bacc.Bacc("TRN2", target_bir_lowering=False, debug=False,
                   num_devices=N_CORES)

    xh = nc.dram_tensor("xh", [128, NDC, S], F8, kind="ExternalInput")
    xl = nc.dram_tensor("xl", [128, NDC, S], F8, kind="ExternalInput")
    wqh = nc.dram_tensor("wqh", [128, NDC, QW], F8, kind="ExternalInput")
    wql = nc.dram_tensor("wql", [128, NDC, QW], F8, kind="ExternalInput")
    wkh = nc.dram_tensor("wkh", [128, NDC, HD], F8, kind="ExternalInput")
    wkl = nc.dram_tensor("wkl", [128, NDC, HD], F8, kind="ExternalInput")
    wvh = nc.dram_tensor("wvh", [128, NDC, HD], F8, kind="ExternalInput")
    wvl = nc.dram_tensor("wvl", [128, NDC, HD], F8, kind="ExternalInput")
    woh = nc.dram_tensor("woh", [128, HPC, D], F8, kind="ExternalInput")
    wol = nc.dram_tensor("wol", [128, HPC, D], F8, kind="ExternalInput")
    cosT = nc.dram_tensor("cosT", [HD, S], BF, kind="ExternalInput")
    sinT = nc.dram_tensor("sinT", [HD, S], BF, kind="ExternalInput")
    rmat = nc.dram_tensor("rmat", [HD, HD], BF, kind="ExternalInput")
    masks = nc.dram_tensor("masks", [128, 512], BF, kind="ExternalInput")
    out = nc.dram_tensor("out", [S, D], BF, kind="ExternalOutput")

    with tile.TileContext(nc) as tc:
        _emit(tc, nc, xh, xl, wqh, wql, wkh, wkl, wvh, wvl, woh, wol,
              cosT, sinT, rmat, masks, out)
    nc.compile()
    return nc


def _emit(tc, nc, xh, xl, wqh, wql, wkh, wkl, wvh, wvl, woh, wol,
          cosT, sinT, rmat, masks, out):
    import contextlib
    ctx = contextlib.ExitStack()
    with ctx:
        const = ctx.enter_context(tc.tile_pool(name="const", bufs=1))
        work = ctx.enter_context(tc.tile_pool(name="work", bufs=1))
        tmp = ctx.enter_context(tc.tile_pool(name="tmp", bufs=4))
        pt_pool = ctx.enter_context(tc.tile_pool(name="pt", bufs=8))
        outp = ctx.enter_context(tc.tile_pool(name="outp", bufs=3))
        ps = ctx.enter_context(tc.tile_pool(name="ps", bufs=8, space="PSUM"))

        # ---- DMA emission, ordered to pace the chunk-major PE stream ----
        rmat_sb = const.tile([HD, HD], BF, tag="rmat")
        ones_sb = const.tile([128, 1], BF, tag="ones")
        nc.vector.memset(ones_sb[:], 1.0)
        ebias = const.tile([128, 1], F32, tag="ebias")
        nc.vector.memset(ebias[:], EXP_SHIFT)

        wk_sb = const.tile([128, 2, NDC, HD], F8, tag="wk")   # dim1: hi/lo
        nc.sync.dma_start(out=rmat_sb[:], in_=rmat.ap())

        gx4h, gx4l = {}, {}
        gqh, gql = {}, {}
        for c in range(0, NDC, 4):
            gx4h[c] = const.tile([128, 4, S], F8, tag=f"xh{c//4}", name=f"xh{c//4}")
            gx4l[c] = const.tile([128, 4, S], F8, tag=f"xl{c//4}", name=f"xl{c//4}")
        gxh = {c: gx4h[c - c % 4][:, c % 4:c % 4 + 2, :] for c in range(0, NDC, 2)}
        gxl = {c: gx4l[c - c % 4][:, c % 4:c % 4 + 2, :] for c in range(0, NDC, 2)}
        # half-0 of x, wq, and wk interleaved in consumption order
        for c in range(0, NDC, 4):
            nc.sync.dma_start(out=wk_sb[:, 0, c:c + 4, :], in_=wkh.ap()[:, c:c + 4, :])
            g = const.tile([128, 4, QW], F8, tag=f"qh{c//4}", name=f"qh{c//4}")
            nc.sync.dma_start(out=g[:], in_=wqh.ap()[:, c:c + 4, :])
            gqh[c], gqh[c + 2] = g[:, 0:2, :], g[:, 2:4, :]
            nc.sync.dma_start(out=gx4h[c][:, :, 0:512], in_=xh.ap()[:, c:c + 4, 0:512])
            nc.sync.dma_start(out=wk_sb[:, 1, c:c + 4, :], in_=wkl.ap()[:, c:c + 4, :])
            g = const.tile([128, 4, QW], F8, tag=f"ql{c//4}", name=f"ql{c//4}")
            nc.sync.dma_start(out=g[:], in_=wql.ap()[:, c:c + 4, :])
            gql[c], gql[c + 2] = g[:, 0:2, :], g[:, 2:4, :]
            nc.sync.dma_start(out=gx4l[c][:, :, 0:512], in_=xl.ap()[:, c:c + 4, 0:512])
        cos_sb = const.tile([HD, S], BF, tag="cos")
        nc.sync.dma_start(out=cos_sb[:], in_=cosT.ap())
        sin_sb = const.tile([HD, S], BF, tag="sin")
        nc.sync.dma_start(out=sin_sb[:], in_=sinT.ap())
        # half-1 of x
        for c in range(0, NDC, 4):
            nc.sync.dma_start(out=gx4h[c][:, :, 512:S], in_=xh.ap()[:, c:c + 4, 512:S])
            nc.sync.dma_start(out=gx4l[c][:, :, 512:S], in_=xl.ap()[:, c:c + 4, 512:S])
        wv_sb = const.tile([128, 2, NDC, HD], F8, tag="wv")
        nc.sync.dma_start(out=wv_sb[:, 0, :, :], in_=wvh.ap())
        nc.sync.dma_start(out=wv_sb[:, 1, :, :], in_=wvl.ap())
        mask_sb = const.tile([128, 512], BF, tag="mask")
        nc.sync.dma_start(out=mask_sb[:], in_=masks.ap())
        wo_sb = const.tile([128, 2, HPC, D], F8, tag="wo")    # dim1: hi/lo
        for n in range(2):
            sl = slice(n * 2048, (n + 1) * 2048)
            nc.sync.dma_start(out=wo_sb[:, 0, :, sl], in_=woh.ap()[:, :, sl])
            nc.sync.dma_start(out=wo_sb[:, 1, :, sl], in_=wol.ap()[:, :, sl])

        # persistent activations
        khat = work.tile([HD, S], BF, tag="khat")
        qhat = [work.tile([HD, S], BF, tag=f"qhat{h}", name=f"qhat{h}")
                for h in range(HPC)]
        v_sb = [work.tile([128, HD], BF, tag=f"v{i}", name=f"v{i}")
                for i in range(8)]
        ctx_hi = [work.tile([128, 2, S], F8, tag=f"cth{u}", name=f"cth{u}")
                  for u in range(2)]
        ctx_lo = [work.tile([128, 2, S], F8, tag=f"ctl{u}", name=f"ctl{u}")
                  for u in range(2)]

        # ---- K+Q projections: chunk-major across 5 chains per s-quarter ----
        # Per chunk pair, all five tensors advance their 3-term DoubleRow
        # chains, so the PE stream follows the x/wq DMA arrival order.
        # RoPE for each finished s-half is queued and its PE/Act/DVE work is
        # injected into later quarters' streams (and the v-projection).
        TENS = [("k", khat, lambda c: wk_sb[:, 0, c:c + 2, :],
                 lambda c: wk_sb[:, 1, c:c + 2, :], 1.0 / (XS * WKS))]
        for h in range(HPC):
            hsl = slice(h * HD, (h + 1) * HD)
            TENS.append((f"q{h}", qhat[h],
                         lambda c, s=hsl: gqh[c][:, :, s],
                         lambda c, s=hsl: gql[c][:, :, s], 1.0 / (XS * WQS)))
        raws = {ti: work.tile([HD, S], BF, tag=f"raw{ti}", name=f"raw{ti}")
                for ti in range(5)}

        pend = []   # queued rope-finish closures (one per (tensor, half))

        def inject_rope():
            if pend:
                pend.pop(0)()

        def rope_half(ti, half):
            name, dst, _, _, _ = TENS[ti]
            sl = slice(half * 512, (half + 1) * 512)
            t1 = tmp.tile([HD, 512], BF, tag="rope_t1", name="rope_t1", bufs=2)
            nc.vector.tensor_mul(t1[:], raws[ti][:, sl], cos_sb[:, sl])
            rq = ps.tile([HD, 512], F32, tag="ps", name="rq")
            nc.tensor.matmul(rq[:], rmat_sb[:], raws[ti][:, sl],
                             start=True, stop=True)
            rqs = tmp.tile([HD, 512], BF, tag="rope_rqs", name="rope_rqs", bufs=2)
            nc.scalar.activation(rqs[:], rq[:],
                                 mybir.ActivationFunctionType.Copy)
            t2 = tmp.tile([HD, 512], BF, tag="rope_t2", name="rope_t2", bufs=2)
            nc.vector.tensor_mul(t2[:], rqs[:], sin_sb[:, sl])
            nc.vector.tensor_add(dst[:, sl], t1[:], t2[:])

        for half in (0, 1):
            hoff = half * 512
            chains = [ps.tile([128, 512], F32, tag="ps", name=f"ch{ti}")
                      for ti in range(5)]
            for p in range(NP):
                c = 2 * p
                for term in range(3):
                    for ti, (_, _, whi, wlo, _) in enumerate(TENS):
                        pp = chains[ti]
                        w = whi(c) if term != 1 else wlo(c)
                        for q in range(2):
                            ssl = slice(hoff + q * 256, hoff + (q + 1) * 256)
                            osl = slice(q * 256, (q + 1) * 256)
                            xop = gxl[c] if term == 2 else gxh[c]
                            nc.tensor.matmul(
                                pp[:, osl], w, xop[:, :, ssl],
                                start=(p == 0 and term == 0 and q == 0),
                                stop=(p == NP - 1 and term == 2 and q == 1),
                                perf_mode=DR, skip_group_check=True)
                if p in (4, 9, 14):
                    inject_rope()
            for ti, (_, _, _, _, descale) in enumerate(TENS):
                nc.scalar.activation(raws[ti][:, hoff:hoff + 512], chains[ti][:],
                                     mybir.ActivationFunctionType.Copy,
                                     scale=descale)
            for ti in range(5):
                pend.append(lambda t=ti, hf=half: rope_half(t, hf))

        # ---- V projection: emitted as PE filler inside early attention ----
        def v_chain(i):
            tsl = slice(i * 128, (i + 1) * 128)
            vp = ps.tile([128, HD], F32, tag="ps", name="vp")
            for p in range(NP):
                c = 2 * p
                nc.tensor.matmul(vp[:], gxh[c][:, :, tsl], wv_sb[:, 0, c:c + 2, :],
                                 start=(p == 0), stop=False, perf_mode=DR)
            for p in range(NP):
                c = 2 * p
                nc.tensor.matmul(vp[:], gxh[c][:, :, tsl], wv_sb[:, 1, c:c + 2, :],
                                 start=False, stop=False, perf_mode=DR)
            for p in range(NP):
                c = 2 * p
                nc.tensor.matmul(vp[:], gxl[c][:, :, tsl], wv_sb[:, 0, c:c + 2, :],
                                 start=False, stop=(p == NP - 1), perf_mode=DR)
            nc.scalar.activation(v_sb[i][:], vp[:],
                                 mybir.ActivationFunctionType.Copy,
                                 scale=1.0 / (XS * WVS))
            inject_rope()

        v_chain(0)
        v_chain(1)
        vq = list(range(2, 8))
        while pend and len(vq) > 4:
            v_chain(vq.pop(0))
        while pend:
            inject_rope()

        # ---- attention + out-proj, software-pipelined ----
        def emit_scores(h, b):
            ssl = slice(b * 256, (b + 1) * 256)
            pts = []
            for tp in range(b + 1):
                st = ps.tile([128, 512], F32, tag="ps", name="st")
                for i in range(2):
                    t0 = (2 * tp + i) * 128
                    nc.tensor.matmul(st[:, i * 256:(i + 1) * 256],
                                     khat[:, t0:t0 + 128], qhat[h][:, ssl],
                                     start=True, stop=True)
                pt = pt_pool.tile([128, 512], BF, tag="pt", name="pt")
                nc.scalar.activation(pt[:], st[:],
                                     mybir.ActivationFunctionType.Exp,
                                     bias=ebias[:])
                if tp == b:
                    nc.vector.tensor_mul(pt[:], pt[:], mask_sb[:])
                pts.append(pt)
            return pts

        def emit_denctx(h, b, pts):
            ssl = slice(b * 256, (b + 1) * 256)
            den = ps.tile([1, 256], F32, tag="ps", name="den")
            n_mm = 2 * (b + 1)
            k = 0
            for pt in pts:
                for i in range(2):
                    nc.tensor.matmul(den[:], ones_sb[:],
                                     pt[:, i * 256:(i + 1) * 256],
                                     start=(k == 0), stop=(k == n_mm - 1))
                    k += 1
            cx = ps.tile([HD, 256], F32, tag="ps", name="cx")
            k = 0
            for tp, pt in enumerate(pts):
                for i in range(2):
                    nc.tensor.matmul(cx[:], v_sb[2 * tp + i][:],
                                     pt[:, i * 256:(i + 1) * 256],
                                     start=(k == 0), stop=(k == n_mm - 1))
                    k += 1
            rec = tmp.tile([1, 256], F32, tag="rec", name="rec", bufs=2)
            nc.vector.reciprocal(rec[:], den[:])
            bc = tmp.tile([128, 256], F32, tag="bc", name="bc", bufs=2)
            nc.gpsimd.partition_broadcast(bc[:], rec[:])
            ctxn = tmp.tile([HD, 256], F32, tag="ctxn", name="ctxn", bufs=2)
            nc.vector.scalar_tensor_tensor(
                ctxn[:], cx[:], CTXS, bc[:],
                op0=mybir.AluOpType.mult, op1=mybir.AluOpType.mult)
            u, par = divmod(h, 2)
            nc.scalar.activation(ctx_hi[u][:, par, ssl], ctxn[:],
                                 mybir.ActivationFunctionType.Copy)
            nc.vector.tensor_sub(ctx_lo[u][:, par, ssl], ctxn[:],
                                 ctx_hi[u][:, par, ssl])

        descale = 1.0 / (CTXS * WOS)

        def emit_outproj(t8):
            tsl = slice(t8 * 128, (t8 + 1) * 128)
            for n4 in range(4):
                ot = outp.tile([128, 1024], BF, tag="ot", name="ot")
                for sub in range(4):
                    n = 4 * n4 + sub
                    nsl = slice(n * 256, (n + 1) * 256)
                    op = ps.tile([128, 256], F32, tag="ps", name="op")
                    k = 0
                    for u in range(2):
                        for chi, whi in ((ctx_hi, 0), (ctx_lo, 0), (ctx_hi, 1)):
                            nc.tensor.matmul(
                                op[:], chi[u][:, :, tsl],
                                wo_sb[:, whi, 2 * u:2 * u + 2, nsl],
                                start=(k == 0), stop=(k == 5), perf_mode=DR)
                            k += 1
                    eng = nc.vector if (sub % 2 == 0) else nc.gpsimd
                    eng.tensor_scalar_mul(ot[:, sub * 256:(sub + 1) * 256],
                                          op[:], descale)
                nc.sync.dma_start(
                    out=out.ap()[tsl, n4 * 1024:(n4 + 1) * 1024], in_=ot[:])

        stages = [(h, b) for b in range(4) for h in range(HPC)]
        prev = None
        outq = []
        for hb in stages:
            pts = emit_scores(*hb)
            if vq:
                v_chain(vq.pop(0))
            if prev is not None:
                (ph, pb), ppts = prev
                emit_denctx(ph, pb, ppts)
                if ph == HPC - 1:
                    outq.extend([2 * pb, 2 * pb + 1])
            if outq:
                emit_outproj(outq.pop(0))
            prev = (hb, pts)
        (ph, pb), ppts = prev
        emit_denctx(ph, pb, ppts)
        outq.extend([2 * pb, 2 * pb + 1])
        for t8 in outq:
            emit_outproj(t8)


def _prep_inputs(x, cos, sin, Wq, Wk, Wv, Wo):
    """Host-side shard + hi/lo fp8 quantization. Returns per-core inputs."""
    bf = ml_dtypes.bfloat16
    f8 = ml_dtypes.float8_e4m3

    def hilo(a, s):
        hi = np.asarray(a * s, np.float32).astype(f8)
        lo = (np.asarray(a * s, np.float32) - hi.astype(np.float32)).astype(f8)
        return hi, lo

    x2 = np.asarray(x, np.float32).reshape(S, D)
    xTh = np.ascontiguousarray(x2.T).reshape(NDC, 128, S).transpose(1, 0, 2)
    xh_, xl_ = hilo(np.ascontiguousarray(xTh), XS)

    cosT = np.ascontiguousarray(np.asarray(cos, np.float32).T).astype(bf)
    sinT = np.ascontiguousarray(np.asarray(sin, np.float32).T).astype(bf)

    rmat = np.zeros((HD, HD), np.float32)
    half = HD // 2
    rmat[np.arange(half), np.arange(half) + half] = 1.0
    rmat[np.arange(half) + half, np.arange(half)] = -1.0
    rmat = rmat.astype(bf)

    # diagonal pair mask: keep when t_local (= i*128 + p) <= s_local
    lt = np.arange(128)[:, None]
    ls = np.arange(256)[None, :]
    masks = np.concatenate([(lt + 128 * i <= ls) for i in range(2)], axis=1)
    masks = np.ascontiguousarray(masks).astype(bf)     # [128, 512]

    scale = 1.0 / np.sqrt(np.float32(HD))
    Wq_ = np.asarray(Wq, np.float32) * scale
    Wk_ = np.asarray(Wk, np.float32)
    Wv_ = np.asarray(Wv, np.float32)
    Wo_ = np.asarray(Wo, np.float32)

    def chunked(w):  # [D, m] -> [128, NDC, m]
        m = w.shape[1]
        return np.ascontiguousarray(
            w.reshape(NDC, 128, m).transpose(1, 0, 2))

    in_maps = []
    for r in range(N_CORES):
        wqh_, wql_ = hilo(chunked(Wq_[:, r * QW:(r + 1) * QW]), WQS)
        wkh_, wkl_ = hilo(chunked(Wk_[:, r * HD:(r + 1) * HD]), WKS)
        wvh_, wvl_ = hilo(chunked(Wv_[:, r * HD:(r + 1) * HD]), WVS)
        wo_r = np.ascontiguousarray(
            Wo_[r * QW:(r + 1) * QW, :].reshape(HPC, 128, D)
            .transpose(1, 0, 2))
        woh_, wol_ = hilo(wo_r, WOS)
        in_maps.append({
            "xh": xh_, "xl": xl_, "wqh": wqh_, "wql": wql_,
            "wkh": wkh_, "wkl": wkl_, "wvh": wvh_, "wvl": wvl_,
            "woh": woh_, "wol": wol_,
            "cosT": cosT, "sinT": sinT, "rmat": rmat, "masks": masks,
        })
    return in_maps


def get_nc():
    if "nc" not in _CACHE:
        _CACHE["nc"] = _build()
    return _CACHE["nc"]


def kernel(x, mask, cos, sin, Wq, Wk, Wv, Wo):
    nc = get_nc()
    in_maps = _prep_inputs(x, cos, sin, Wq, Wk, Wv, Wo)
    res = run_bass_kernel_spmd(nc, in_maps, core_ids=list(range(N_CORES)))
    acc = np.zeros((S, D), np.float32)
    for r in range(N_CORES):
        acc += res.results[r]["out"].astype(np.float32)
    return acc[None]


if __name__ == "__main__":
    print("built:", get_nc() is not None)
